# revision 23
# baseline (speedup 1.0000x reference)
"""Two-layer GAT (4-head then 1-head) on 8 NeuronCores.

Sharding: nodes are partitioned across the 8 cores by dst-ownership
(6272 = 49*128 aligned nodes per core).  Each core processes all edges whose
dst it owns.  Per-dst-window (128 nodes) the segment softmax + weighted
aggregation run as one-hot-selection matmuls on the tensor engine.

Edge-gathered features are stored d-major ([64][4heads] interleaved) so the
per-edge softmax weight applies as ONE stride-0-broadcast DVE multiply at
2-byte speed (no broadcast materialization, no big scalar-engine pass).

Three SPMD launches:
  K1: h|el|er = x @ [W0^T | vl0^T | vr0^T]   (node-sharded, bf16)
  K2: L0 edge phase (attention + aggregation) + relu + g|el1|er1 matmul
  K3: L1 edge phase -> output

Between launches the host performs pure index gathers / dtype casts of
device-computed tables; all floating-point math runs on device.
"""
import os
import sys
import types

sys.path.insert(0, "/opt/trn_rl_repo")

import numpy as np

import concourse.bass as bass
import concourse.tile as tile
from concourse import mybir
from concourse.bass_utils import run_bass_kernel_spmd
from concourse.vector_clock import ScopedClock

# ---------------------------------------------------------------- constants
N_NODES = int(os.environ.get("GAT_N_NODES", "50000"))
IN_F = 256
HID = 64
HEADS = 4
OUT_F = 64
NEG_SLOPE = 0.2

NC_CORES = 8
P = 128
W_PER_CORE = int(os.environ.get("GAT_W", "49"))
OWN = W_PER_CORE * P            # 6272 nodes per core
PADN = NC_CORES * OWN           # 50176
F32 = mybir.dt.float32
BF16 = mybir.dt.bfloat16
F8 = mybir.dt.float8e4

EXEC_TIMES_NS = {}              # filled when GAT_PROFILE=1


# ------------------------------------------------------------- tile patches
def _patch_tile():
    """This container's walrus rejects instructions with >1 sem wait
    ("Too many sync wait commands").  After Tile lowering, move excess waits
    onto same-engine no-ops inserted before the offending instruction."""
    if getattr(_patch_tile, "done", False):
        return
    _patch_tile.done = True

    MAX_WAITS = 1

    def _split_all_waits(nc):
        for bb in nc.main_func.blocks:
            insts = bb.instructions
            i = 0
            while i < len(insts):
                inst = insts[i]
                si = getattr(inst, "sync_info", None)
                if si is None or len(si.on_wait) <= MAX_WAITS:
                    i += 1
                    continue
                waits = list(si.on_wait)
                si.on_wait[:] = waits[:MAX_WAITS]
                extra = waits[MAX_WAITS:]
                nops = []
                for j in range(0, len(extra), MAX_WAITS):
                    nop = mybir.InstNoOp(
                        name=f"I-waitsplit-{nc.next_id()}",
                        ins=[],
                        outs=[],
                        engine=inst.engine,
                    )
                    nop.sync_info = mybir.SyncInfo(
                        on_wait=extra[j : j + MAX_WAITS], on_update=[]
                    )
                    nc.register_instruction(nop, overwrite=True)
                    nops.append(nop)
                insts[i:i] = nops
                i += len(nops) + 1

    def _drain_and_barrier(self, tick_clock, wait_clock):
        drain_inst = self.nc.sync.drain()
        wait_clock.add_sem_waits(
            drain_inst.ins, ScopedClock({None: tick_clock.global_clock})
        )
        self.nc.all_engine_barrier()
        assert self.sems is not None
        popped = self.nc._tile_sem_poison_stack.pop()
        assert popped is self._sem_poison
        self.nc.clear_and_free_semaphores(list(self.sems.allocated().values()))
        self.nc.all_engine_barrier()
        _split_all_waits(self.nc)

    tile.TileContext._drain_and_barrier = _drain_and_barrier


def _install_ntff_hook():
    """Enable run_bass_kernel_spmd(trace=True) under axon: register the NTFF
    profile hook that the boot script skips when antenv.axon_hooks is absent."""
    if getattr(_install_ntff_hook, "done", False):
        return
    _install_ntff_hook.done = True
    try:
        mod = types.ModuleType("antenv.axon_hooks")
        _state = {}

        def set_axon_ntff_profile_hook(h):
            _state["h"] = h

        def get_axon_ntff_profile_hook():
            return _state.get("h")

        mod.set_axon_ntff_profile_hook = set_axon_ntff_profile_hook
        mod.get_axon_ntff_profile_hook = get_axon_ntff_profile_hook
        sys.modules["antenv.axon_hooks"] = mod
        import antenv

        antenv.axon_hooks = mod
        from trn_agent_boot.trn_boot import _ntff_profile_via_ctypes

        hook = _ntff_profile_via_ctypes("/opt/axon/libaxon_pjrt.so")
        if hook is not None:
            set_axon_ntff_profile_hook(hook)
    except Exception:
        pass


# ------------------------------------------------------------- kernel builders
def build_k1():
    """h (bf16) | el,er (f32) tables for this core's 6272 nodes.

    Streams x in 7-window chunks and the output tables back out per chunk so
    DMA, PE, and the psum-evacuation copies overlap instead of serializing.
    """
    nc = bass.Bass()
    DE = IN_F + 2 * HEADS                     # 264
    CH = 7                                    # windows per chunk
    NCH = W_PER_CORE // CH                    # 7 chunks
    assert CH * NCH == W_PER_CORE
    xT_own = nc.dram_tensor("xT_own", [IN_F, OWN], BF16, kind="ExternalInput")
    w0te = nc.dram_tensor("w0te", [IN_F, DE], BF16, kind="ExternalInput")
    hb = nc.dram_tensor("hb", [OWN, IN_F], BF16, kind="ExternalOutput")
    elr = nc.dram_tensor("elr", [OWN, 2 * HEADS], F32, kind="ExternalOutput")

    with tile.TileContext(nc) as tc:
        with (
            tc.tile_pool(name="const", bufs=1) as constp,
            tc.tile_pool(name="sbuf", bufs=3) as pool,
            tc.tile_pool(name="psum", bufs=4, space="PSUM") as psum,
        ):
            wt = constp.tile([P, 2, DE], BF16)
            nc.sync.dma_start(wt[:, 0, :], w0te[0:P, :])
            nc.sync.dma_start(wt[:, 1, :], w0te[P : 2 * P, :])
            for ci in range(NCH):
                lo = ci * CH * P
                xkc = pool.tile([P, 2, CH * P], BF16, tag="xkc")
                nc.sync.dma_start(xkc[:, 0, :], xT_own[0:P, lo : lo + CH * P])
                nc.sync.dma_start(xkc[:, 1, :], xT_own[P : 2 * P, lo : lo + CH * P])
                hbc = pool.tile([P, CH, IN_F], BF16, tag="hbc")
                elrc = pool.tile([P, CH, 2 * HEADS], F32, tag="elrc")
                for mi in range(CH):
                    pu = psum.tile([P, DE], F32, tag="pu")
                    for kk in range(2):
                        nc.tensor.matmul(
                            pu[:],
                            lhsT=xkc[:, kk, mi * P : (mi + 1) * P],
                            rhs=wt[:, kk, :],
                            start=(kk == 0),
                            stop=(kk == 1),
                        )
                    nc.scalar.copy(hbc[:, mi, :], pu[:, 0:IN_F])
                    nc.vector.tensor_copy(elrc[:, mi, :], pu[:, IN_F:DE])
                hbd = hb.rearrange("(w p) f -> p w f", p=P)
                elrd = elr.rearrange("(w p) f -> p w f", p=P)
                nc.sync.dma_start(hbd[:, ci * CH : (ci + 1) * CH, :], hbc[:])
                nc.scalar.dma_start(elrd[:, ci * CH : (ci + 1) * CH, :], elrc[:])
    return nc


def build_k2(C, use_eps, has_b0):
    """L0 edge phase + relu + L1 node matmul.

    Inputs (per core):
      h_edge [W, P, C*256] bf16  gathered h rows (src), d-major (d,h) interleave
      meta   [W, P, C*8]   bf16  per chunk: el(4) | er(4)
      S_in   [W, P, C*128] fp8   one-hot dst-lane selectors
      b0r    [P, 256]      bf16  (d,h)-interleaved b0 rows
      ident  [P, 128]      bf16
      w1te   [256, 66]     bf16  rows (d,h)-interleaved
    Outputs:
      gb  [OWN, 64] bf16   g rows for this core's nodes
      e1  [OWN, 2]  f32    el1 | er1
    """
    nc = bass.Bass()
    HF = HEADS * HID                           # 256
    G = OUT_F + 2                              # 66
    RW = HF + 4                                # 260 msg row width
    h_edge = nc.dram_tensor("h_edge", [W_PER_CORE, P, C * HF], BF16, kind="ExternalInput")
    meta = nc.dram_tensor("meta", [W_PER_CORE, P, C * 8], BF16, kind="ExternalInput")
    S_in = nc.dram_tensor("S_in", [W_PER_CORE, P, C * 128], F8, kind="ExternalInput")
    b0r = nc.dram_tensor("b0r", [P, HF], BF16, kind="ExternalInput")
    ident_t = nc.dram_tensor("ident", [P, 128], BF16, kind="ExternalInput")
    w1te = nc.dram_tensor("w1te", [HF, G], BF16, kind="ExternalInput")
    gb = nc.dram_tensor("gb", [OWN, OUT_F], BF16, kind="ExternalOutput")
    e1 = nc.dram_tensor("e1", [OWN, 2], F32, kind="ExternalOutput")

    with tile.TileContext(nc) as tc:
        with (
            tc.tile_pool(name="const", bufs=1) as constp,
            tc.tile_pool(name="sbuf", bufs=4) as pool,
            tc.tile_pool(name="small", bufs=4) as spool,
            tc.tile_pool(name="psum", bufs=4, space="PSUM") as psum,
            tc.tile_pool(name="psum2", bufs=2, space="PSUM") as psum2,
        ):
            b0_sb = constp.tile([P, HF], BF16)
            nc.sync.dma_start(b0_sb[:], b0r[:])
            ident_sb = constp.tile([P, 128], BF16)
            nc.sync.dma_start(ident_sb[:], ident_t[:])
            w1_sb = constp.tile([P, 2, G], BF16)
            nc.sync.dma_start(w1_sb[:, 0, :], w1te[0:P, :])
            nc.sync.dma_start(w1_sb[:, 1, :], w1te[P : 2 * P, :])
            h1_all = constp.tile([P, W_PER_CORE * HF], BF16)
            gb_all = constp.tile([P, W_PER_CORE, OUT_F], BF16)
            e1_all = constp.tile([P, W_PER_CORE, 2], F32)

            SKEW = 2          # back-stage lags front-stage by 2 windows
            pus = {}          # window -> live psum tile

            def front(w):
                he = pool.tile([P, C, HID, HEADS], BF16, tag="he")
                nc.sync.dma_start(
                    he[:], h_edge[w].rearrange("p (c d h) -> p c d h", d=HID, h=HEADS)
                )
                S_all = pool.tile([P, C, 128], F8, tag="S_all")
                nc.sync.dma_start(S_all[:], S_in[w].rearrange("p (c n) -> p c n", n=128))
                mt = pool.tile([P, C, 8], BF16, tag="mt")
                nc.sync.dma_start(
                    mt[:], meta[w].rearrange("p (c n) -> p c n", n=8)
                )

                # e = lrelu(el + er) = 0.8*relu(el+er) + 0.2*(el+er)  (all gpsimd)
                e = spool.tile([P, C, HEADS], BF16, tag="e")
                nc.gpsimd.tensor_tensor(
                    out=e[:], in0=mt[:, :, 0:4], in1=mt[:, :, 4:8],
                    op=mybir.AluOpType.add,
                )
                t = spool.tile([P, C, HEADS], BF16, tag="t")
                nc.gpsimd.tensor_scalar_max(t[:], e[:], 0.0)
                nc.gpsimd.tensor_scalar_mul(t[:], t[:], 1.0 - NEG_SLOPE)
                nc.gpsimd.tensor_scalar_mul(e[:], e[:], NEG_SLOPE)
                nc.gpsimd.tensor_tensor(
                    out=e[:], in0=e[:], in1=t[:], op=mybir.AluOpType.add
                )
                ee = spool.tile([P, C, HEADS], BF16, tag="ee")
                nc.scalar.activation(ee[:], e[:], mybir.ActivationFunctionType.Exp)

                # msg[p, c, 0:256](d,h) = he * ee (bcast mult at 2x, split into
                # two tiles so the PE can start on the first chunks early)
                # msg[p, c, 256:260]    = ee      (denominator columns)
                ee4 = ee[:].rearrange("p c (o h) -> p c o h", o=1, h=HEADS)
                CH0 = C // 2
                msgs = []
                for tag, clo, chi in (("msga", 0, CH0), ("msgb", CH0, C)):
                    cw = chi - clo
                    m = pool.tile([P, cw, RW], BF16, tag=tag)
                    msgs.append(m)
                    nc.vector.tensor_tensor(
                        out=m[:, :, 0:HF].rearrange(
                            "p c (d h) -> p c d h", d=HID, h=HEADS
                        ),
                        in0=he[:, clo:chi],
                        in1=ee4[:, clo:chi].to_broadcast([P, cw, HID, HEADS]),
                        op=mybir.AluOpType.mult,
                    )
                    nc.scalar.activation(
                        m[:, :, HF : HF + 4], e[:, clo:chi],
                        mybir.ActivationFunctionType.Exp,
                    )

                pu = psum.tile([P, RW], F32, tag="pu")
                for c in range(C):
                    m = msgs[0] if c < CH0 else msgs[1]
                    nc.tensor.matmul(
                        pu[:], lhsT=S_all[:, c, :], rhs=m[:, c - (0 if c < CH0 else CH0), :],
                        start=(c == 0), stop=(c == C - 1),
                    )
                pus[w] = pu

            def back(w):
                pu = pus.pop(w)
                # rs = 1/sum(ee) per head (bf16), h1 = relu(num*rs) (+b0)
                rs_b = spool.tile([P, 1, HEADS], BF16, tag="rs_b")
                with nc.allow_low_precision(reason="1/sum(ee) applied to bf16 msg"):
                    if use_eps:
                        s_eps = spool.tile([P, HEADS], F32, tag="s_eps")
                        nc.vector.tensor_scalar_add(s_eps[:], pu[:, HF : HF + 4], 1e-38)
                        nc.vector.reciprocal(rs_b[:, 0, :], s_eps[:])
                    else:
                        nc.vector.reciprocal(rs_b[:, 0, :], pu[:, HF : HF + 4])
                h1w = h1_all[:, w * HF : (w + 1) * HF]
                h1w4 = h1w.rearrange("p (d h) -> p d h", d=HID, h=HEADS)
                if has_b0:
                    # general path: h1 = relu(num*rs + b0)
                    nc.scalar.copy(h1w, pu[:, 0:HF])
                    nc.vector.tensor_tensor(
                        out=h1w4, in0=h1w4,
                        in1=rs_b[:].to_broadcast([P, HID, HEADS]),
                        op=mybir.AluOpType.mult,
                    )
                    nc.vector.tensor_tensor(
                        out=h1w, in0=h1w, in1=b0_sb[:], op=mybir.AluOpType.add
                    )
                    nc.vector.tensor_scalar_max(h1w, h1w, 0.0)
                else:
                    # b0 == 0: relu(num)*rs == relu(num*rs); fold relu into
                    # the psum evacuation on the scalar engine
                    nc.scalar.activation(
                        h1w, pu[:, 0:HF], mybir.ActivationFunctionType.Relu
                    )
                    nc.vector.tensor_tensor(
                        out=h1w4, in0=h1w4,
                        in1=rs_b[:].to_broadcast([P, HID, HEADS]),
                        op=mybir.AluOpType.mult,
                    )

                # L1 node matmul for this window: g|el1|er1 = relu_h1 @ w1te
                pg = psum2.tile([P, G], F32, tag="pg")
                for kk in range(2):
                    pt = psum2.tile([P, 128], BF16, tag="pt")
                    nc.tensor.transpose(
                        out=pt[:],
                        in_=h1_all[:, w * HF + kk * P : w * HF + (kk + 1) * P],
                        identity=ident_sb[:],
                    )
                    h1t = spool.tile([P, 128], BF16, tag="h1t")
                    nc.scalar.copy(h1t[:], pt[:])
                    nc.tensor.matmul(
                        pg[:], lhsT=h1t[:], rhs=w1_sb[:, kk, :],
                        start=(kk == 0), stop=(kk == 1),
                    )
                nc.scalar.copy(gb_all[:, w, :], pg[:, 0:OUT_F])
                nc.vector.tensor_copy(e1_all[:, w, :], pg[:, OUT_F:G])

            for w in range(W_PER_CORE + SKEW):
                if w < W_PER_CORE:
                    front(w)
                if w >= SKEW:
                    back(w - SKEW)
            nc.sync.dma_start(gb.rearrange("(w p) f -> p w f", p=P), gb_all[:])
            nc.sync.dma_start(e1.rearrange("(w p) f -> p w f", p=P), e1_all[:])
    return nc


def build_k3(C, use_eps, has_b1):
    """L1 edge phase: y = (sum_e ee1*g[src]) / (sum_e ee1) + b1 per dst node."""
    nc = bass.Bass()
    g_edge = nc.dram_tensor("g_edge", [W_PER_CORE, P, C * OUT_F], BF16, kind="ExternalInput")
    meta1 = nc.dram_tensor("meta1", [W_PER_CORE, P, C * 2], BF16, kind="ExternalInput")
    S_in = nc.dram_tensor("S_in", [W_PER_CORE, P, C * 128], F8, kind="ExternalInput")
    b1r = nc.dram_tensor("b1r", [P, OUT_F], F32, kind="ExternalInput")
    y_out = nc.dram_tensor("y_out", [OWN, OUT_F], F32, kind="ExternalOutput")
    RW = OUT_F + 1                             # 65: msg | ee

    WPAIR = (W_PER_CORE + 1) // 2

    with tile.TileContext(nc) as tc:
        with (
            tc.tile_pool(name="const", bufs=1) as constp,
            tc.tile_pool(name="sbuf", bufs=4) as pool,
            tc.tile_pool(name="small", bufs=4) as spool,
            tc.tile_pool(name="psum", bufs=4, space="PSUM") as psum,
        ):
            b1_sb = constp.tile([P, OUT_F], F32)
            nc.sync.dma_start(b1_sb[:], b1r[:])
            y_all = constp.tile([P, W_PER_CORE, OUT_F], F32)

            SKEW = 2
            pus = {}
            tiles = {}

            def front(w):
                if w % 2 == 0:
                    nw = 2 if w + 1 < W_PER_CORE else 1
                    ge = pool.tile([P, 2, C, OUT_F], BF16, tag="ge")
                    nc.sync.dma_start(
                        ge[:, 0:nw],
                        g_edge[w : w + nw].rearrange("w p (c f) -> p w c f", f=OUT_F),
                    )
                    S_all = pool.tile([P, 2, C, 128], F8, tag="S_all")
                    nc.sync.dma_start(
                        S_all[:, 0:nw],
                        S_in[w : w + nw].rearrange("w p (c n) -> p w c n", n=128),
                    )
                    mt = pool.tile([P, 2, C, 2], BF16, tag="mt")
                    nc.scalar.dma_start(
                        mt[:, 0:nw],
                        meta1[w : w + nw].rearrange("w p (c n) -> p w c n", n=2),
                    )
                    tiles[w // 2] = (ge, S_all, mt)
                ge, S_all, mt = tiles[w // 2]
                wi = w % 2
                # e = lrelu(el + er) = 0.8*relu(e) + 0.2*e  (all gpsimd)
                e = spool.tile([P, C, 1], BF16, tag="e")
                nc.gpsimd.tensor_tensor(
                    out=e[:], in0=mt[:, wi, :, 0:1], in1=mt[:, wi, :, 1:2],
                    op=mybir.AluOpType.add,
                )
                t = spool.tile([P, C, 1], BF16, tag="t")
                nc.gpsimd.tensor_scalar_max(t[:], e[:], 0.0)
                nc.gpsimd.tensor_scalar_mul(t[:], t[:], 1.0 - NEG_SLOPE)
                nc.gpsimd.tensor_scalar_mul(e[:], e[:], NEG_SLOPE)
                nc.gpsimd.tensor_tensor(
                    out=e[:], in0=e[:], in1=t[:], op=mybir.AluOpType.add
                )
                ee = spool.tile([P, C, 1], BF16, tag="ee")
                nc.scalar.activation(ee[:], e[:], mybir.ActivationFunctionType.Exp)

                CH0 = C // 2
                msgs = []
                for tag, clo, chi in (("msga", 0, CH0), ("msgb", CH0, C)):
                    cw = chi - clo
                    m = pool.tile([P, cw, RW], BF16, tag=tag)
                    msgs.append(m)
                    nc.vector.tensor_tensor(
                        out=m[:, :, 0:OUT_F], in0=ge[:, wi, clo:chi],
                        in1=ee[:, clo:chi].to_broadcast([P, cw, OUT_F]),
                        op=mybir.AluOpType.mult,
                    )
                    nc.scalar.activation(
                        m[:, :, OUT_F : OUT_F + 1], e[:, clo:chi],
                        mybir.ActivationFunctionType.Exp,
                    )

                pu = psum.tile([P, RW], F32, tag="pu")
                for c in range(C):
                    m = msgs[0] if c < CH0 else msgs[1]
                    nc.tensor.matmul(
                        pu[:], lhsT=S_all[:, wi, c, :],
                        rhs=m[:, c - (0 if c < CH0 else CH0), :],
                        start=(c == 0), stop=(c == C - 1),
                    )
                pus[w] = pu

            def back(w):
                pu = pus.pop(w)
                rs = spool.tile([P, 1], F32, tag="rs")
                if use_eps:
                    s_eps = spool.tile([P, 1], F32, tag="s_eps")
                    nc.vector.tensor_scalar_add(
                        s_eps[:], pu[:, OUT_F : OUT_F + 1], 1e-38
                    )
                    nc.vector.reciprocal(rs[:], s_eps[:])
                else:
                    nc.vector.reciprocal(rs[:], pu[:, OUT_F : OUT_F + 1])
                yw = y_all[:, w, :]
                # y = num * rs (+ b1): scalar Copy activation with scale
                nc.scalar.activation(
                    yw, pu[:, 0:OUT_F],
                    mybir.ActivationFunctionType.Copy, scale=rs[:, 0:1],
                )
                if has_b1:
                    nc.vector.tensor_tensor(
                        out=yw, in0=yw, in1=b1_sb[:], op=mybir.AluOpType.add
                    )

            for w in range(W_PER_CORE + SKEW):
                if w < W_PER_CORE:
                    front(w)
                if w >= SKEW:
                    back(w - SKEW)
            nc.sync.dma_start(y_out.rearrange("(w p) f -> p w f", p=P), y_all[:])
    return nc


# ------------------------------------------------------------- host helpers
def _run(nc, in_maps, label):
    profile = os.environ.get("GAT_PROFILE", "0") == "1"
    res = run_bass_kernel_spmd(
        nc, in_maps, core_ids=list(range(NC_CORES)), trace=profile
    )
    if profile:
        EXEC_TIMES_NS[label] = res.exec_time_ns
    return res.results


def _edge_slots(src, dst):
    """Per-core edge->slot assignment.  Returns (C, sidx, ddst, dloc, min_deg)."""
    core = dst // OWN
    win = (dst - core * OWN) // P
    loc = (dst - core * OWN) % P

    counts = np.zeros((NC_CORES, W_PER_CORE), dtype=np.int64)
    np.add.at(counts, (core, win), 1)
    C = int(np.ceil(counts.max() / P))

    deg = np.bincount(dst, minlength=PADN)[:N_NODES]
    min_deg = int(deg.min())

    order = np.lexsort((win, core))
    s_src, s_core, s_win, s_loc = src[order], core[order], win[order], loc[order]
    group = s_core * W_PER_CORE + s_win
    gstart = np.zeros(NC_CORES * W_PER_CORE, dtype=np.int64)
    cnt = np.bincount(group, minlength=NC_CORES * W_PER_CORE)
    gstart[1:] = np.cumsum(cnt)[:-1]
    within = np.arange(len(order)) - gstart[group]

    sidx = np.full((NC_CORES, W_PER_CORE, C * P), -1, dtype=np.int64)
    ddst = np.full((NC_CORES, W_PER_CORE, C * P), -1, dtype=np.int64)
    dloc = np.full((NC_CORES, W_PER_CORE, C * P), -1.0, dtype=np.float32)
    sidx[s_core, s_win, within] = s_src
    ddst[s_core, s_win, within] = s_core * OWN + s_win * P + s_loc
    dloc[s_core, s_win, within] = s_loc.astype(np.float32)
    return C, sidx, ddst, dloc, min_deg


def _to_tiles(rows, C, ncol):
    """[W, C*P, ncol] -> [W, P, C*ncol] (slot j -> partition j%P, chunk j//P)."""
    W = rows.shape[0]
    return (
        rows.reshape(W, C, P, ncol).transpose(0, 2, 1, 3).reshape(W, P, C * ncol)
    )


def kernel(x, src, dst, W0, al0, ar0, b0, W1, al1, ar1, b1):
    _patch_tile()
    _install_ntff_hook()

    x = np.asarray(x, dtype=np.float32)
    src = np.asarray(src, dtype=np.int64)
    dst = np.asarray(dst, dtype=np.int64)
    W0 = np.asarray(W0, dtype=np.float32)
    al0 = np.asarray(al0, dtype=np.float32)
    ar0 = np.asarray(ar0, dtype=np.float32)
    b0 = np.asarray(b0, dtype=np.float32)
    W1 = np.asarray(W1, dtype=np.float32)
    al1 = np.asarray(al1, dtype=np.float32)
    ar1 = np.asarray(ar1, dtype=np.float32)
    b1 = np.asarray(b1, dtype=np.float32)

    HF = HEADS * HID
    G = OUT_F + 2

    import ml_dtypes

    BF = ml_dtypes.bfloat16
    F8H = ml_dtypes.float8_e4m3

    # (d,h)-interleave permutation over the 256 hidden columns
    il = (np.arange(HF).reshape(HEADS, HID).T).reshape(-1)   # il[d*4+h] = h*64+d

    # ---- weight prep
    vl0 = np.einsum("hd,hdk->hk", al0, W0.reshape(HEADS, HID, IN_F))   # [4, 256]
    vr0 = np.einsum("hd,hdk->hk", ar0, W0.reshape(HEADS, HID, IN_F))
    w0te = np.concatenate([W0.T, vl0.T, vr0.T], axis=1).astype(BF)     # [256, 264]
    vl1 = al1 @ W1                                                      # [1, 256]
    vr1 = ar1 @ W1
    w1te = np.concatenate([W1.T, vl1.T, vr1.T], axis=1)[il].astype(BF)  # [256, 66] permuted rows

    xT_pad = np.zeros((IN_F, PADN), dtype=BF)
    xT_pad[:, :N_NODES] = x.T.astype(BF)

    ident = np.eye(128, dtype=BF)
    b0r = np.tile(b0[il][None, :], (P, 1)).astype(BF)
    b1r = np.tile(b1[None, :], (P, 1)).astype(np.float32)
    has_b0 = bool(np.any(b0))
    has_b1 = bool(np.any(b1))

    # ---- K1: node tables
    nc1 = build_k1()
    in1 = [
        {"xT_own": np.ascontiguousarray(xT_pad[:, k * OWN : (k + 1) * OWN]), "w0te": w0te}
        for k in range(NC_CORES)
    ]
    r1 = _run(nc1, in1, "k1")
    hbtab = np.concatenate([r1[k]["hb"] for k in range(NC_CORES)], axis=0)   # [PADN,256] bf16
    elrtab = np.concatenate([r1[k]["elr"] for k in range(NC_CORES)], axis=0)  # [PADN,8] f32

    # ---- edge layout
    C, sidx, ddst, dloc, min_deg = _edge_slots(src, dst)
    use_eps = min_deg == 0

    # pre-permute table columns to (d,h)-interleaved order once
    hbtab_il = np.ascontiguousarray(hbtab[:, il])
    hbtab_x = np.concatenate([hbtab_il, np.zeros((1, IN_F), dtype=BF)], axis=0)
    elrtab_x = np.concatenate([elrtab, np.zeros((1, 8), dtype=np.float32)], axis=0)

    s_cl = np.where(sidx < 0, PADN, sidx)
    d_cl = np.where(ddst < 0, PADN, ddst)

    # one-hot tiles: S[w, p, c*128+n] = (dstloc == n)
    def s_tiles(dl):
        oh = (dl.reshape(W_PER_CORE, C, P)[:, :, :, None]
              == np.arange(128, dtype=np.float32)[None, None, None, :])
        return np.ascontiguousarray(
            oh.transpose(0, 2, 1, 3).reshape(W_PER_CORE, P, C * 128).astype(F8H)
        )

    # ---- K2 inputs
    nc2 = build_k2(C, use_eps, has_b0)
    in2 = []
    for k in range(NC_CORES):
        h_edge = _to_tiles(hbtab_x[s_cl[k], :], C, HF)          # bf16 byte gather
        meta = np.empty((W_PER_CORE, C * P, 8), dtype=np.float32)
        meta[:, :, 0:4] = elrtab_x[s_cl[k], 0:4]
        meta[:, :, 4:8] = elrtab_x[d_cl[k], 4:8]
        meta = _to_tiles(meta, C, 8).astype(BF)
        in2.append(
            {
                "h_edge": np.ascontiguousarray(h_edge),
                "meta": np.ascontiguousarray(meta),
                "S_in": s_tiles(dloc[k]),
                "b0r": b0r,
                "ident": ident,
                "w1te": w1te,
            }
        )
    r2 = _run(nc2, in2, "k2")
    gbtab = np.concatenate([r2[k]["gb"] for k in range(NC_CORES)], axis=0)   # [PADN,64] bf16
    e1tab = np.concatenate([r2[k]["e1"] for k in range(NC_CORES)], axis=0)   # [PADN,2] f32
    gbtab_x = np.concatenate([gbtab, np.zeros((1, OUT_F), dtype=BF)], axis=0)
    e1tab_x = np.concatenate([e1tab, np.zeros((1, 2), dtype=np.float32)], axis=0)

    # ---- K3 inputs
    nc3 = build_k3(C, use_eps, has_b1)
    in3 = []
    for k in range(NC_CORES):
        g_edge = _to_tiles(gbtab_x[s_cl[k], :], C, OUT_F)       # bf16 byte gather
        meta1 = np.empty((W_PER_CORE, C * P, 2), dtype=np.float32)
        meta1[:, :, 0] = e1tab_x[s_cl[k], 0]
        meta1[:, :, 1] = e1tab_x[d_cl[k], 1]
        meta1 = _to_tiles(meta1, C, 2).astype(BF)
        in3.append(
            {
                "g_edge": np.ascontiguousarray(g_edge),
                "meta1": np.ascontiguousarray(meta1),
                "S_in": in2[k]["S_in"],
                "b1r": b1r,
            }
        )
    r3 = _run(nc3, in3, "k3")
    y = np.concatenate([r3[k]["y_out"] for k in range(NC_CORES)], axis=0)
    return np.ascontiguousarray(y[:N_NODES]).astype(np.float32)


# revision 32
# speedup vs baseline: 1.0532x; 1.0532x over previous
"""Two-layer GAT (4-head then 1-head) on 8 NeuronCores.

Sharding: nodes are partitioned across the 8 cores by dst-ownership
(6272 = 49*128 aligned nodes per core).  Each core processes all edges whose
dst it owns.  Per-dst-window (128 nodes) the segment softmax + weighted
aggregation run as one-hot-selection matmuls on the tensor engine.

Edge-gathered features are stored d-major ([64][4heads] interleaved) so the
per-edge softmax weight applies as ONE stride-0-broadcast DVE multiply at
2-byte speed (no broadcast materialization, no big scalar-engine pass).

Three SPMD launches:
  K1: h|el|er = x @ [W0^T | vl0^T | vr0^T]   (node-sharded, bf16)
  K2: L0 edge phase (attention + aggregation) + relu + g|el1|er1 matmul
  K3: L1 edge phase -> output

Between launches the host performs pure index gathers / dtype casts of
device-computed tables; all floating-point math runs on device.
"""
import os
import sys
import types

sys.path.insert(0, "/opt/trn_rl_repo")

import numpy as np

import concourse.bass as bass
import concourse.tile as tile
from concourse import mybir
from concourse.bass_utils import run_bass_kernel_spmd
from concourse.vector_clock import ScopedClock

# ---------------------------------------------------------------- constants
N_NODES = int(os.environ.get("GAT_N_NODES", "50000"))
IN_F = 256
HID = 64
HEADS = 4
OUT_F = 64
NEG_SLOPE = 0.2

NC_CORES = 8
P = 128
W_PER_CORE = int(os.environ.get("GAT_W", "49"))
OWN = W_PER_CORE * P            # 6272 nodes per core
PADN = NC_CORES * OWN           # 50176
F32 = mybir.dt.float32
BF16 = mybir.dt.bfloat16
F8 = mybir.dt.float8e4

EXEC_TIMES_NS = {}              # filled when GAT_PROFILE=1


# ------------------------------------------------------------- tile patches
def _patch_tile():
    """This container's walrus rejects instructions with >1 sem wait
    ("Too many sync wait commands").  After Tile lowering, move excess waits
    onto same-engine no-ops inserted before the offending instruction."""
    if getattr(_patch_tile, "done", False):
        return
    _patch_tile.done = True

    MAX_WAITS = 1

    def _split_all_waits(nc):
        for bb in nc.main_func.blocks:
            insts = bb.instructions
            i = 0
            while i < len(insts):
                inst = insts[i]
                si = getattr(inst, "sync_info", None)
                if si is None or len(si.on_wait) <= MAX_WAITS:
                    i += 1
                    continue
                waits = list(si.on_wait)
                si.on_wait[:] = waits[:MAX_WAITS]
                extra = waits[MAX_WAITS:]
                nops = []
                for j in range(0, len(extra), MAX_WAITS):
                    nop = mybir.InstNoOp(
                        name=f"I-waitsplit-{nc.next_id()}",
                        ins=[],
                        outs=[],
                        engine=inst.engine,
                    )
                    nop.sync_info = mybir.SyncInfo(
                        on_wait=extra[j : j + MAX_WAITS], on_update=[]
                    )
                    nc.register_instruction(nop, overwrite=True)
                    nops.append(nop)
                insts[i:i] = nops
                i += len(nops) + 1

    def _drain_and_barrier(self, tick_clock, wait_clock):
        drain_inst = self.nc.sync.drain()
        wait_clock.add_sem_waits(
            drain_inst.ins, ScopedClock({None: tick_clock.global_clock})
        )
        self.nc.all_engine_barrier()
        assert self.sems is not None
        popped = self.nc._tile_sem_poison_stack.pop()
        assert popped is self._sem_poison
        self.nc.clear_and_free_semaphores(list(self.sems.allocated().values()))
        self.nc.all_engine_barrier()
        _split_all_waits(self.nc)

    tile.TileContext._drain_and_barrier = _drain_and_barrier


def _install_ntff_hook():
    """Enable run_bass_kernel_spmd(trace=True) under axon: register the NTFF
    profile hook that the boot script skips when antenv.axon_hooks is absent."""
    if getattr(_install_ntff_hook, "done", False):
        return
    _install_ntff_hook.done = True
    try:
        mod = types.ModuleType("antenv.axon_hooks")
        _state = {}

        def set_axon_ntff_profile_hook(h):
            _state["h"] = h

        def get_axon_ntff_profile_hook():
            return _state.get("h")

        mod.set_axon_ntff_profile_hook = set_axon_ntff_profile_hook
        mod.get_axon_ntff_profile_hook = get_axon_ntff_profile_hook
        sys.modules["antenv.axon_hooks"] = mod
        import antenv

        antenv.axon_hooks = mod
        from trn_agent_boot.trn_boot import _ntff_profile_via_ctypes

        hook = _ntff_profile_via_ctypes("/opt/axon/libaxon_pjrt.so")
        if hook is not None:
            set_axon_ntff_profile_hook(hook)
    except Exception:
        pass


# ------------------------------------------------------------- kernel builders
def build_k1():
    """h (bf16) | el,er (f32) tables for this core's 6272 nodes.

    Streams x in 7-window chunks and the output tables back out per chunk so
    DMA, PE, and the psum-evacuation copies overlap instead of serializing.
    """
    nc = bass.Bass()
    DE = IN_F + 2 * HEADS                     # 264
    CH = 7                                    # windows per chunk
    NCH = W_PER_CORE // CH                    # 7 chunks
    assert CH * NCH == W_PER_CORE
    xT_own = nc.dram_tensor("xT_own", [IN_F, OWN], BF16, kind="ExternalInput")
    w0te = nc.dram_tensor("w0te", [IN_F, DE], BF16, kind="ExternalInput")
    hb = nc.dram_tensor("hb", [OWN, IN_F], BF16, kind="ExternalOutput")
    elr = nc.dram_tensor("elr", [OWN, 2 * HEADS], F32, kind="ExternalOutput")

    with tile.TileContext(nc) as tc:
        with (
            tc.tile_pool(name="const", bufs=1) as constp,
            tc.tile_pool(name="sbuf", bufs=3) as pool,
            tc.tile_pool(name="psum", bufs=4, space="PSUM") as psum,
        ):
            wt = constp.tile([P, 2, DE], BF16)
            nc.sync.dma_start(wt[:, 0, :], w0te[0:P, :])
            nc.sync.dma_start(wt[:, 1, :], w0te[P : 2 * P, :])
            for ci in range(NCH):
                lo = ci * CH * P
                xkc = pool.tile([P, 2, CH * P], BF16, tag="xkc")
                nc.sync.dma_start(xkc[:, 0, :], xT_own[0:P, lo : lo + CH * P])
                nc.sync.dma_start(xkc[:, 1, :], xT_own[P : 2 * P, lo : lo + CH * P])
                hbc = pool.tile([P, CH, IN_F], BF16, tag="hbc")
                elrc = pool.tile([P, CH, 2 * HEADS], F32, tag="elrc")
                for mi in range(CH):
                    pu = psum.tile([P, DE], F32, tag="pu")
                    for kk in range(2):
                        nc.tensor.matmul(
                            pu[:],
                            lhsT=xkc[:, kk, mi * P : (mi + 1) * P],
                            rhs=wt[:, kk, :],
                            start=(kk == 0),
                            stop=(kk == 1),
                        )
                    nc.scalar.copy(hbc[:, mi, :], pu[:, 0:IN_F])
                    nc.vector.tensor_copy(elrc[:, mi, :], pu[:, IN_F:DE])
                hbd = hb.rearrange("(w p) f -> p w f", p=P)
                elrd = elr.rearrange("(w p) f -> p w f", p=P)
                nc.sync.dma_start(hbd[:, ci * CH : (ci + 1) * CH, :], hbc[:])
                nc.scalar.dma_start(elrd[:, ci * CH : (ci + 1) * CH, :], elrc[:])
    return nc


def build_k2(C, use_eps, has_b0):
    """L0 edge phase + relu + L1 node matmul.

    Inputs (per core):
      h_edge [W, P, C*256] bf16  gathered h rows (src), d-major (d,h) interleave
      meta   [W, P, C*8]   bf16  per chunk: el(4) | er(4)
      S_in   [W, P, C*128] fp8   one-hot dst-lane selectors
      b0r    [P, 256]      bf16  (d,h)-interleaved b0 rows
      ident  [P, 128]      bf16
      w1te   [256, 66]     bf16  rows (d,h)-interleaved
    Outputs:
      gb  [OWN, 64] bf16   g rows for this core's nodes
      e1  [OWN, 2]  f32    el1 | er1
    """
    nc = bass.Bass()
    HF = HEADS * HID                           # 256
    G = OUT_F + 2                              # 66
    RW = HF + 4                                # 260 msg row width = (64+1 d)*(4 h)
    DD = HID + 1                               # 65: d rows plus ones-row per head
    h_edge = nc.dram_tensor("h_edge", [W_PER_CORE, P, C * RW], BF16, kind="ExternalInput")
    meta = nc.dram_tensor("meta", [W_PER_CORE, P, C * 8], BF16, kind="ExternalInput")
    S_in = nc.dram_tensor("S_in", [W_PER_CORE, P, C * 128], F8, kind="ExternalInput")
    b0r = nc.dram_tensor("b0r", [P, HF], BF16, kind="ExternalInput")
    ident_t = nc.dram_tensor("ident", [P, 128], BF16, kind="ExternalInput")
    w1te = nc.dram_tensor("w1te", [HF, G], BF16, kind="ExternalInput")
    gb = nc.dram_tensor("gb", [OWN, OUT_F], BF16, kind="ExternalOutput")
    e1 = nc.dram_tensor("e1", [OWN, 2], F32, kind="ExternalOutput")

    with tile.TileContext(nc) as tc:
        with (
            tc.tile_pool(name="const", bufs=1) as constp,
            tc.tile_pool(name="sbuf", bufs=4) as pool,
            tc.tile_pool(name="small", bufs=4) as spool,
            tc.tile_pool(name="psum", bufs=4, space="PSUM") as psum,
            tc.tile_pool(name="psum2", bufs=2, space="PSUM") as psum2,
        ):
            b0_sb = constp.tile([P, HF], BF16)
            nc.sync.dma_start(b0_sb[:], b0r[:])
            ident_sb = constp.tile([P, 128], BF16)
            nc.sync.dma_start(ident_sb[:], ident_t[:])
            w1_sb = constp.tile([P, 2, G], BF16)
            nc.sync.dma_start(w1_sb[:, 0, :], w1te[0:P, :])
            nc.sync.dma_start(w1_sb[:, 1, :], w1te[P : 2 * P, :])
            h1_all = constp.tile([P, W_PER_CORE * HF], BF16)
            gb_all = constp.tile([P, W_PER_CORE, OUT_F], BF16)
            e1_all = constp.tile([P, W_PER_CORE, 2], F32)

            SKEW = 2          # back-stage lags front-stage by 2 windows
            pus = {}          # window -> live psum tile

            def front(w):
                he = pool.tile([P, C, DD, HEADS], BF16, tag="he")
                nc.sync.dma_start(
                    he[:], h_edge[w].rearrange("p (c d h) -> p c d h", d=DD, h=HEADS)
                )
                S_all = pool.tile([P, C, 128], F8, tag="S_all")
                nc.sync.dma_start(S_all[:], S_in[w].rearrange("p (c n) -> p c n", n=128))
                mt = pool.tile([P, C, 8], BF16, tag="mt")
                nc.sync.dma_start(
                    mt[:], meta[w].rearrange("p (c n) -> p c n", n=8)
                )

                # e = lrelu(el + er)  [P, C, 4]
                e = spool.tile([P, C, HEADS], BF16, tag="e")
                nc.gpsimd.tensor_tensor(
                    out=e[:], in0=mt[:, :, 0:4], in1=mt[:, :, 4:8],
                    op=mybir.AluOpType.add,
                )
                t = spool.tile([P, C, HEADS], BF16, tag="t")
                nc.gpsimd.tensor_scalar_mul(t[:], e[:], NEG_SLOPE)
                nc.vector.tensor_tensor(
                    out=e[:], in0=e[:], in1=t[:], op=mybir.AluOpType.max
                )
                ee = spool.tile([P, C, HEADS], BF16, tag="ee")
                nc.scalar.activation(ee[:], e[:], mybir.ActivationFunctionType.Exp)

                # msg[p, c, (d,h)] = he_ext * ee: he carries a ones-row per
                # head (d=64), so one bcast mult yields numerator AND
                # denominator columns. Split into two tiles so the PE can
                # start on the first chunks early.
                ee4 = ee[:].rearrange("p c (o h) -> p c o h", o=1, h=HEADS)
                CH0 = C // 2
                msgs = []
                for tag, clo, chi in (("msga", 0, CH0), ("msgb", CH0, C)):
                    cw = chi - clo
                    m = pool.tile([P, cw, RW], BF16, tag=tag)
                    msgs.append(m)
                    nc.vector.tensor_tensor(
                        out=m[:].rearrange(
                            "p c (d h) -> p c d h", d=DD, h=HEADS
                        ),
                        in0=he[:, clo:chi],
                        in1=ee4[:, clo:chi].to_broadcast([P, cw, DD, HEADS]),
                        op=mybir.AluOpType.mult,
                    )

                pu = psum.tile([P, RW], F32, tag="pu")
                for c in range(C):
                    m = msgs[0] if c < CH0 else msgs[1]
                    nc.tensor.matmul(
                        pu[:], lhsT=S_all[:, c, :], rhs=m[:, c - (0 if c < CH0 else CH0), :],
                        start=(c == 0), stop=(c == C - 1),
                    )
                pus[w] = pu

            def back(w):
                pu = pus.pop(w)
                # rs = 1/sum(ee) per head (bf16), h1 = relu(num*rs) (+b0)
                rs_b = spool.tile([P, 1, HEADS], BF16, tag="rs_b")
                with nc.allow_low_precision(reason="1/sum(ee) applied to bf16 msg"):
                    if use_eps:
                        s_eps = spool.tile([P, HEADS], F32, tag="s_eps")
                        nc.vector.tensor_scalar_add(s_eps[:], pu[:, HF : HF + 4], 1e-38)
                        nc.vector.reciprocal(rs_b[:, 0, :], s_eps[:])
                    else:
                        nc.vector.reciprocal(rs_b[:, 0, :], pu[:, HF : HF + 4])
                h1w = h1_all[:, w * HF : (w + 1) * HF]
                h1w4 = h1w.rearrange("p (d h) -> p d h", d=HID, h=HEADS)
                if has_b0:
                    # general path: h1 = relu(num*rs + b0)
                    nc.scalar.copy(h1w, pu[:, 0:HF])
                    nc.vector.tensor_tensor(
                        out=h1w4, in0=h1w4,
                        in1=rs_b[:].to_broadcast([P, HID, HEADS]),
                        op=mybir.AluOpType.mult,
                    )
                    nc.vector.tensor_tensor(
                        out=h1w, in0=h1w, in1=b0_sb[:], op=mybir.AluOpType.add
                    )
                    nc.vector.tensor_scalar_max(h1w, h1w, 0.0)
                else:
                    # b0 == 0: relu(num)*rs == relu(num*rs); fold relu into
                    # the psum evacuation on the scalar engine
                    nc.scalar.activation(
                        h1w, pu[:, 0:HF], mybir.ActivationFunctionType.Relu
                    )
                    nc.vector.tensor_tensor(
                        out=h1w4, in0=h1w4,
                        in1=rs_b[:].to_broadcast([P, HID, HEADS]),
                        op=mybir.AluOpType.mult,
                    )

                # L1 node matmul for this window: g|el1|er1 = relu_h1 @ w1te
                pg = psum2.tile([P, G], F32, tag="pg")
                for kk in range(2):
                    pt = psum2.tile([P, 128], BF16, tag="pt")
                    nc.tensor.transpose(
                        out=pt[:],
                        in_=h1_all[:, w * HF + kk * P : w * HF + (kk + 1) * P],
                        identity=ident_sb[:],
                    )
                    h1t = spool.tile([P, 128], BF16, tag="h1t")
                    nc.scalar.copy(h1t[:], pt[:])
                    nc.tensor.matmul(
                        pg[:], lhsT=h1t[:], rhs=w1_sb[:, kk, :],
                        start=(kk == 0), stop=(kk == 1),
                    )
                nc.scalar.copy(gb_all[:, w, :], pg[:, 0:OUT_F])
                nc.vector.tensor_copy(e1_all[:, w, :], pg[:, OUT_F:G])

            for w in range(W_PER_CORE + SKEW):
                if w < W_PER_CORE:
                    front(w)
                if w >= SKEW:
                    back(w - SKEW)
            nc.sync.dma_start(gb.rearrange("(w p) f -> p w f", p=P), gb_all[:])
            nc.sync.dma_start(e1.rearrange("(w p) f -> p w f", p=P), e1_all[:])
    return nc


def build_k3(C, use_eps, has_b1):
    """L1 edge phase: y = (sum_e ee1*g[src]) / (sum_e ee1) + b1 per dst node."""
    nc = bass.Bass()
    g_edge = nc.dram_tensor("g_edge", [W_PER_CORE, P, C * (OUT_F + 1)], BF16, kind="ExternalInput")
    meta1 = nc.dram_tensor("meta1", [W_PER_CORE, P, C * 2], BF16, kind="ExternalInput")
    S_in = nc.dram_tensor("S_in", [W_PER_CORE, P, C * 128], F8, kind="ExternalInput")
    b1r = nc.dram_tensor("b1r", [P, OUT_F], F32, kind="ExternalInput")
    y_out = nc.dram_tensor("y_out", [OWN, OUT_F], F32, kind="ExternalOutput")
    RW = OUT_F + 1                             # 65: msg | ee

    WPAIR = (W_PER_CORE + 1) // 2

    with tile.TileContext(nc) as tc:
        with (
            tc.tile_pool(name="const", bufs=1) as constp,
            tc.tile_pool(name="sbuf", bufs=4) as pool,
            tc.tile_pool(name="small", bufs=4) as spool,
            tc.tile_pool(name="psum", bufs=4, space="PSUM") as psum,
        ):
            b1_sb = constp.tile([P, OUT_F], F32)
            nc.sync.dma_start(b1_sb[:], b1r[:])
            y_all = constp.tile([P, W_PER_CORE, OUT_F], F32)

            SKEW = 2
            pus = {}
            tiles = {}

            def front(w):
                if w % 2 == 0:
                    nw = 2 if w + 1 < W_PER_CORE else 1
                    ge = pool.tile([P, 2, C, RW], BF16, tag="ge")
                    nc.sync.dma_start(
                        ge[:, 0:nw],
                        g_edge[w : w + nw].rearrange("w p (c f) -> p w c f", f=RW),
                    )
                    S_all = pool.tile([P, 2, C, 128], F8, tag="S_all")
                    nc.sync.dma_start(
                        S_all[:, 0:nw],
                        S_in[w : w + nw].rearrange("w p (c n) -> p w c n", n=128),
                    )
                    mt = pool.tile([P, 2, C, 2], BF16, tag="mt")
                    nc.scalar.dma_start(
                        mt[:, 0:nw],
                        meta1[w : w + nw].rearrange("w p (c n) -> p w c n", n=2),
                    )
                    tiles[w // 2] = (ge, S_all, mt)
                ge, S_all, mt = tiles[w // 2]
                wi = w % 2
                # e = lrelu(el + er), ee = exp(e)
                e = spool.tile([P, C, 1], BF16, tag="e")
                nc.gpsimd.tensor_tensor(
                    out=e[:], in0=mt[:, wi, :, 0:1], in1=mt[:, wi, :, 1:2],
                    op=mybir.AluOpType.add,
                )
                t = spool.tile([P, C, 1], BF16, tag="t")
                nc.gpsimd.tensor_scalar_mul(t[:], e[:], NEG_SLOPE)
                nc.vector.tensor_tensor(
                    out=e[:], in0=e[:], in1=t[:], op=mybir.AluOpType.max
                )
                ee = spool.tile([P, C, 1], BF16, tag="ee")
                nc.scalar.activation(ee[:], e[:], mybir.ActivationFunctionType.Exp)

                # ge carries a ones-column (f=64): one bcast mult yields
                # numerator and denominator together
                CH0 = C // 2
                msgs = []
                for tag, clo, chi in (("msga", 0, CH0), ("msgb", CH0, C)):
                    cw = chi - clo
                    m = pool.tile([P, cw, RW], BF16, tag=tag)
                    msgs.append(m)
                    nc.vector.tensor_tensor(
                        out=m[:], in0=ge[:, wi, clo:chi],
                        in1=ee[:, clo:chi].to_broadcast([P, cw, RW]),
                        op=mybir.AluOpType.mult,
                    )

                pu = psum.tile([P, RW], F32, tag="pu")
                for c in range(C):
                    m = msgs[0] if c < CH0 else msgs[1]
                    nc.tensor.matmul(
                        pu[:], lhsT=S_all[:, wi, c, :],
                        rhs=m[:, c - (0 if c < CH0 else CH0), :],
                        start=(c == 0), stop=(c == C - 1),
                    )
                pus[w] = pu

            def back(w):
                pu = pus.pop(w)
                rs = spool.tile([P, 1], F32, tag="rs")
                if use_eps:
                    s_eps = spool.tile([P, 1], F32, tag="s_eps")
                    nc.vector.tensor_scalar_add(
                        s_eps[:], pu[:, OUT_F : OUT_F + 1], 1e-38
                    )
                    nc.vector.reciprocal(rs[:], s_eps[:])
                else:
                    nc.vector.reciprocal(rs[:], pu[:, OUT_F : OUT_F + 1])
                yw = y_all[:, w, :]
                # y = num * rs (+ b1): scalar Copy activation with scale
                nc.scalar.activation(
                    yw, pu[:, 0:OUT_F],
                    mybir.ActivationFunctionType.Copy, scale=rs[:, 0:1],
                )
                if has_b1:
                    nc.vector.tensor_tensor(
                        out=yw, in0=yw, in1=b1_sb[:], op=mybir.AluOpType.add
                    )

            for w in range(W_PER_CORE + SKEW):
                if w < W_PER_CORE:
                    front(w)
                if w >= SKEW:
                    back(w - SKEW)
            nc.sync.dma_start(y_out.rearrange("(w p) f -> p w f", p=P), y_all[:])
    return nc


# ------------------------------------------------------------- host helpers
def _run(nc, in_maps, label):
    profile = os.environ.get("GAT_PROFILE", "0") == "1"
    res = run_bass_kernel_spmd(
        nc, in_maps, core_ids=list(range(NC_CORES)), trace=profile
    )
    if profile:
        EXEC_TIMES_NS[label] = res.exec_time_ns
    return res.results


def _edge_slots(src, dst):
    """Per-core edge->slot assignment.  Returns (C, sidx, ddst, dloc, min_deg)."""
    core = dst // OWN
    win = (dst - core * OWN) // P
    loc = (dst - core * OWN) % P

    counts = np.zeros((NC_CORES, W_PER_CORE), dtype=np.int64)
    np.add.at(counts, (core, win), 1)
    C = int(np.ceil(counts.max() / P))

    deg = np.bincount(dst, minlength=PADN)[:N_NODES]
    min_deg = int(deg.min())

    order = np.lexsort((win, core))
    s_src, s_core, s_win, s_loc = src[order], core[order], win[order], loc[order]
    group = s_core * W_PER_CORE + s_win
    gstart = np.zeros(NC_CORES * W_PER_CORE, dtype=np.int64)
    cnt = np.bincount(group, minlength=NC_CORES * W_PER_CORE)
    gstart[1:] = np.cumsum(cnt)[:-1]
    within = np.arange(len(order)) - gstart[group]

    sidx = np.full((NC_CORES, W_PER_CORE, C * P), -1, dtype=np.int64)
    ddst = np.full((NC_CORES, W_PER_CORE, C * P), -1, dtype=np.int64)
    dloc = np.full((NC_CORES, W_PER_CORE, C * P), -1.0, dtype=np.float32)
    sidx[s_core, s_win, within] = s_src
    ddst[s_core, s_win, within] = s_core * OWN + s_win * P + s_loc
    dloc[s_core, s_win, within] = s_loc.astype(np.float32)
    return C, sidx, ddst, dloc, min_deg


def _to_tiles(rows, C, ncol):
    """[W, C*P, ncol] -> [W, P, C*ncol] (slot j -> partition j%P, chunk j//P)."""
    W = rows.shape[0]
    return (
        rows.reshape(W, C, P, ncol).transpose(0, 2, 1, 3).reshape(W, P, C * ncol)
    )


def kernel(x, src, dst, W0, al0, ar0, b0, W1, al1, ar1, b1):
    _patch_tile()
    _install_ntff_hook()

    x = np.asarray(x, dtype=np.float32)
    src = np.asarray(src, dtype=np.int64)
    dst = np.asarray(dst, dtype=np.int64)
    W0 = np.asarray(W0, dtype=np.float32)
    al0 = np.asarray(al0, dtype=np.float32)
    ar0 = np.asarray(ar0, dtype=np.float32)
    b0 = np.asarray(b0, dtype=np.float32)
    W1 = np.asarray(W1, dtype=np.float32)
    al1 = np.asarray(al1, dtype=np.float32)
    ar1 = np.asarray(ar1, dtype=np.float32)
    b1 = np.asarray(b1, dtype=np.float32)

    HF = HEADS * HID
    G = OUT_F + 2

    import ml_dtypes

    BF = ml_dtypes.bfloat16
    F8H = ml_dtypes.float8_e4m3

    # (d,h)-interleave permutation over the 256 hidden columns
    il = (np.arange(HF).reshape(HEADS, HID).T).reshape(-1)   # il[d*4+h] = h*64+d

    # ---- weight prep
    vl0 = np.einsum("hd,hdk->hk", al0, W0.reshape(HEADS, HID, IN_F))   # [4, 256]
    vr0 = np.einsum("hd,hdk->hk", ar0, W0.reshape(HEADS, HID, IN_F))
    w0te = np.concatenate([W0.T, vl0.T, vr0.T], axis=1).astype(BF)     # [256, 264]
    vl1 = al1 @ W1                                                      # [1, 256]
    vr1 = ar1 @ W1
    w1te = np.concatenate([W1.T, vl1.T, vr1.T], axis=1)[il].astype(BF)  # [256, 66] permuted rows

    xT_pad = np.zeros((IN_F, PADN), dtype=BF)
    xT_pad[:, :N_NODES] = x.T.astype(BF)

    ident = np.eye(128, dtype=BF)
    b0r = np.tile(b0[il][None, :], (P, 1)).astype(BF)
    b1r = np.tile(b1[None, :], (P, 1)).astype(np.float32)
    has_b0 = bool(np.any(b0))
    has_b1 = bool(np.any(b1))

    # ---- K1: node tables
    nc1 = build_k1()
    in1 = [
        {"xT_own": np.ascontiguousarray(xT_pad[:, k * OWN : (k + 1) * OWN]), "w0te": w0te}
        for k in range(NC_CORES)
    ]
    r1 = _run(nc1, in1, "k1")
    hbtab = np.concatenate([r1[k]["hb"] for k in range(NC_CORES)], axis=0)   # [PADN,256] bf16
    elrtab = np.concatenate([r1[k]["elr"] for k in range(NC_CORES)], axis=0)  # [PADN,8] f32

    # ---- edge layout
    C, sidx, ddst, dloc, min_deg = _edge_slots(src, dst)
    use_eps = min_deg == 0

    # (d,h)-interleaved h table with a ones-row per head (d=64) so the device
    # multiply produces numerator and softmax denominator in one op
    hb3 = hbtab.reshape(PADN, HEADS, HID).transpose(0, 2, 1)       # [N, 64, 4]
    hb_ext = np.concatenate([hb3, np.ones((PADN, 1, HEADS), dtype=BF)], axis=1)
    hbtab_x = np.concatenate(
        [hb_ext.reshape(PADN, (HID + 1) * HEADS),
         np.zeros((1, (HID + 1) * HEADS), dtype=BF)], axis=0
    )
    elrtab_x = np.concatenate([elrtab, np.zeros((1, 8), dtype=np.float32)], axis=0)

    s_cl = np.where(sidx < 0, PADN, sidx)
    d_cl = np.where(ddst < 0, PADN, ddst)

    # one-hot tiles: S[w, p, c*128+n] = (dstloc == n)
    def s_tiles(dl):
        oh = (dl.reshape(W_PER_CORE, C, P)[:, :, :, None]
              == np.arange(128, dtype=np.float32)[None, None, None, :])
        return np.ascontiguousarray(
            oh.transpose(0, 2, 1, 3).reshape(W_PER_CORE, P, C * 128).astype(F8H)
        )

    # ---- K2 inputs
    nc2 = build_k2(C, use_eps, has_b0)
    in2 = []
    for k in range(NC_CORES):
        h_edge = _to_tiles(hbtab_x[s_cl[k], :], C, HF + 4)      # bf16 byte gather
        meta = np.empty((W_PER_CORE, C * P, 8), dtype=np.float32)
        meta[:, :, 0:4] = elrtab_x[s_cl[k], 0:4]
        meta[:, :, 4:8] = elrtab_x[d_cl[k], 4:8]
        meta = _to_tiles(meta, C, 8).astype(BF)
        in2.append(
            {
                "h_edge": np.ascontiguousarray(h_edge),
                "meta": np.ascontiguousarray(meta),
                "S_in": s_tiles(dloc[k]),
                "b0r": b0r,
                "ident": ident,
                "w1te": w1te,
            }
        )
    r2 = _run(nc2, in2, "k2")
    gbtab = np.concatenate([r2[k]["gb"] for k in range(NC_CORES)], axis=0)   # [PADN,64] bf16
    e1tab = np.concatenate([r2[k]["e1"] for k in range(NC_CORES)], axis=0)   # [PADN,2] f32
    gb_ext = np.concatenate([gbtab, np.ones((PADN, 1), dtype=BF)], axis=1)
    gbtab_x = np.concatenate([gb_ext, np.zeros((1, OUT_F + 1), dtype=BF)], axis=0)
    e1tab_x = np.concatenate([e1tab, np.zeros((1, 2), dtype=np.float32)], axis=0)

    # ---- K3 inputs
    nc3 = build_k3(C, use_eps, has_b1)
    in3 = []
    for k in range(NC_CORES):
        g_edge = _to_tiles(gbtab_x[s_cl[k], :], C, OUT_F + 1)   # bf16 byte gather
        meta1 = np.empty((W_PER_CORE, C * P, 2), dtype=np.float32)
        meta1[:, :, 0] = e1tab_x[s_cl[k], 0]
        meta1[:, :, 1] = e1tab_x[d_cl[k], 1]
        meta1 = _to_tiles(meta1, C, 2).astype(BF)
        in3.append(
            {
                "g_edge": np.ascontiguousarray(g_edge),
                "meta1": np.ascontiguousarray(meta1),
                "S_in": in2[k]["S_in"],
                "b1r": b1r,
            }
        )
    r3 = _run(nc3, in3, "k3")
    y = np.concatenate([r3[k]["y_out"] for k in range(NC_CORES)], axis=0)
    return np.ascontiguousarray(y[:N_NODES]).astype(np.float32)


# revision 42
# speedup vs baseline: 1.2033x; 1.1425x over previous
"""Two-layer GAT (4-head then 1-head) on 8 NeuronCores.

Sharding: nodes are partitioned across the 8 cores by dst-ownership
(6272 = 49*128 aligned nodes per core).  Each core processes all edges whose
dst it owns.  Per-dst-window (128 nodes) the segment softmax + weighted
aggregation run as one-hot-selection matmuls on the tensor engine.

Edge-gathered features are stored d-major ([64][4heads] interleaved) so the
per-edge softmax weight applies as ONE stride-0-broadcast DVE multiply at
2-byte speed (no broadcast materialization, no big scalar-engine pass).

Three SPMD launches:
  K1: h|el|er = x @ [W0^T | vl0^T | vr0^T]   (node-sharded, bf16)
  K2: L0 edge phase (attention + aggregation) + relu + g|el1|er1 matmul
  K3: L1 edge phase -> output

Between launches the host performs pure index gathers / dtype casts of
device-computed tables; all floating-point math runs on device.
"""
import os
import sys
import types

sys.path.insert(0, "/opt/trn_rl_repo")

import numpy as np

import concourse.bass as bass
import concourse.tile as tile
from concourse import mybir
from concourse.bass_utils import run_bass_kernel_spmd
from concourse.vector_clock import ScopedClock

# ---------------------------------------------------------------- constants
N_NODES = int(os.environ.get("GAT_N_NODES", "50000"))
IN_F = 256
HID = 64
HEADS = 4
OUT_F = 64
NEG_SLOPE = 0.2

NC_CORES = 8
P = 128
W_PER_CORE = int(os.environ.get("GAT_W", "49"))
OWN = W_PER_CORE * P            # 6272 nodes per core
PADN = NC_CORES * OWN           # 50176
F32 = mybir.dt.float32
BF16 = mybir.dt.bfloat16
F8 = mybir.dt.float8e4

EXEC_TIMES_NS = {}              # filled when GAT_PROFILE=1


# ------------------------------------------------------------- tile patches
def _patch_tile():
    """This container's walrus rejects instructions with >1 sem wait
    ("Too many sync wait commands").  After Tile lowering, move excess waits
    onto same-engine no-ops inserted before the offending instruction."""
    if getattr(_patch_tile, "done", False):
        return
    _patch_tile.done = True

    MAX_WAITS = 1

    def _split_all_waits(nc):
        for bb in nc.main_func.blocks:
            insts = bb.instructions
            i = 0
            while i < len(insts):
                inst = insts[i]
                si = getattr(inst, "sync_info", None)
                if si is None or len(si.on_wait) <= MAX_WAITS:
                    i += 1
                    continue
                waits = list(si.on_wait)
                si.on_wait[:] = waits[:MAX_WAITS]
                extra = waits[MAX_WAITS:]
                nops = []
                for j in range(0, len(extra), MAX_WAITS):
                    nop = mybir.InstNoOp(
                        name=f"I-waitsplit-{nc.next_id()}",
                        ins=[],
                        outs=[],
                        engine=inst.engine,
                    )
                    nop.sync_info = mybir.SyncInfo(
                        on_wait=extra[j : j + MAX_WAITS], on_update=[]
                    )
                    nc.register_instruction(nop, overwrite=True)
                    nops.append(nop)
                insts[i:i] = nops
                i += len(nops) + 1

    def _drain_and_barrier(self, tick_clock, wait_clock):
        drain_inst = self.nc.sync.drain()
        wait_clock.add_sem_waits(
            drain_inst.ins, ScopedClock({None: tick_clock.global_clock})
        )
        self.nc.all_engine_barrier()
        assert self.sems is not None
        popped = self.nc._tile_sem_poison_stack.pop()
        assert popped is self._sem_poison
        self.nc.clear_and_free_semaphores(list(self.sems.allocated().values()))
        self.nc.all_engine_barrier()
        _split_all_waits(self.nc)

    tile.TileContext._drain_and_barrier = _drain_and_barrier


def _install_ntff_hook():
    """Enable run_bass_kernel_spmd(trace=True) under axon: register the NTFF
    profile hook that the boot script skips when antenv.axon_hooks is absent."""
    if getattr(_install_ntff_hook, "done", False):
        return
    _install_ntff_hook.done = True
    try:
        mod = types.ModuleType("antenv.axon_hooks")
        _state = {}

        def set_axon_ntff_profile_hook(h):
            _state["h"] = h

        def get_axon_ntff_profile_hook():
            return _state.get("h")

        mod.set_axon_ntff_profile_hook = set_axon_ntff_profile_hook
        mod.get_axon_ntff_profile_hook = get_axon_ntff_profile_hook
        sys.modules["antenv.axon_hooks"] = mod
        import antenv

        antenv.axon_hooks = mod
        from trn_agent_boot.trn_boot import _ntff_profile_via_ctypes

        hook = _ntff_profile_via_ctypes("/opt/axon/libaxon_pjrt.so")
        if hook is not None:
            set_axon_ntff_profile_hook(hook)
    except Exception:
        pass


# ------------------------------------------------------------- kernel builders
def build_k1():
    """h (bf16) | el,er (f32) tables for this core's 6272 nodes.

    Streams x in 7-window chunks and the output tables back out per chunk so
    DMA, PE, and the psum-evacuation copies overlap instead of serializing.
    """
    nc = bass.Bass()
    DE = IN_F + 2 * HEADS                     # 264
    CH = 7                                    # windows per chunk
    NCH = W_PER_CORE // CH                    # 7 chunks
    assert CH * NCH == W_PER_CORE
    xT_own = nc.dram_tensor("xT_own", [IN_F, OWN], BF16, kind="ExternalInput")
    w0te = nc.dram_tensor("w0te", [IN_F, DE], BF16, kind="ExternalInput")
    hb = nc.dram_tensor("hb", [OWN, IN_F], BF16, kind="ExternalOutput")
    elr = nc.dram_tensor("elr", [OWN, 2 * HEADS], F32, kind="ExternalOutput")

    with tile.TileContext(nc) as tc:
        with (
            tc.tile_pool(name="const", bufs=1) as constp,
            tc.tile_pool(name="sbuf", bufs=3) as pool,
            tc.tile_pool(name="psum", bufs=4, space="PSUM") as psum,
        ):
            wt = constp.tile([P, 2, DE], BF16)
            nc.sync.dma_start(wt[:, 0, :], w0te[0:P, :])
            nc.sync.dma_start(wt[:, 1, :], w0te[P : 2 * P, :])
            for ci in range(NCH):
                lo = ci * CH * P
                xkc = pool.tile([P, 2, CH * P], BF16, tag="xkc")
                nc.sync.dma_start(xkc[:, 0, :], xT_own[0:P, lo : lo + CH * P])
                nc.sync.dma_start(xkc[:, 1, :], xT_own[P : 2 * P, lo : lo + CH * P])
                hbc = pool.tile([P, CH, IN_F], BF16, tag="hbc")
                elrc = pool.tile([P, CH, 2 * HEADS], F32, tag="elrc")
                for mi in range(CH):
                    pu = psum.tile([P, DE], F32, tag="pu")
                    for kk in range(2):
                        nc.tensor.matmul(
                            pu[:],
                            lhsT=xkc[:, kk, mi * P : (mi + 1) * P],
                            rhs=wt[:, kk, :],
                            start=(kk == 0),
                            stop=(kk == 1),
                        )
                    nc.scalar.copy(hbc[:, mi, :], pu[:, 0:IN_F])
                    nc.vector.tensor_copy(elrc[:, mi, :], pu[:, IN_F:DE])
                hbd = hb.rearrange("(w p) f -> p w f", p=P)
                elrd = elr.rearrange("(w p) f -> p w f", p=P)
                nc.sync.dma_start(hbd[:, ci * CH : (ci + 1) * CH, :], hbc[:])
                nc.scalar.dma_start(elrd[:, ci * CH : (ci + 1) * CH, :], elrc[:])
    return nc


def build_k2(C, use_eps, has_b0):
    """L0 edge phase + relu + L1 node matmul.

    Inputs (per core):
      h_edge [W, P, C*256] bf16  gathered h rows (src), d-major (d,h) interleave
      meta   [W, P, C*8]   bf16  per chunk: el(4) | er(4)
      S_in   [W, P, C*128] fp8   one-hot dst-lane selectors
      b0r    [P, 256]      bf16  (d,h)-interleaved b0 rows
      ident  [P, 128]      bf16
      w1te   [256, 66]     bf16  rows (d,h)-interleaved
    Outputs:
      gb  [OWN, 64] bf16   g rows for this core's nodes
      e1  [OWN, 2]  f32    el1 | er1
    """
    nc = bass.Bass()
    HF = HEADS * HID                           # 256
    G = OUT_F + 2                              # 66
    RW = HF + 4                                # 260 msg row width
    h_edge = nc.dram_tensor("h_edge", [W_PER_CORE, P, C * HF], BF16, kind="ExternalInput")
    meta = nc.dram_tensor("meta", [W_PER_CORE, P, C * 8], BF16, kind="ExternalInput")
    S_in = nc.dram_tensor("S_in", [W_PER_CORE, P, C * 128], F8, kind="ExternalInput")
    b0r = nc.dram_tensor("b0r", [P, HF], BF16, kind="ExternalInput")
    ident_t = nc.dram_tensor("ident", [P, 128], BF16, kind="ExternalInput")
    w1te = nc.dram_tensor("w1te", [HF, G], BF16, kind="ExternalInput")
    gb = nc.dram_tensor("gb", [OWN, OUT_F], BF16, kind="ExternalOutput")
    e1 = nc.dram_tensor("e1", [OWN, 2], F32, kind="ExternalOutput")

    with tile.TileContext(nc) as tc:
        with (
            tc.tile_pool(name="const", bufs=1) as constp,
            tc.tile_pool(name="sbuf", bufs=4) as pool,
            tc.tile_pool(name="small", bufs=4) as spool,
            tc.tile_pool(name="psum", bufs=4, space="PSUM") as psum,
            tc.tile_pool(name="psum2", bufs=2, space="PSUM") as psum2,
        ):
            b0_sb = constp.tile([P, HF], BF16)
            nc.sync.dma_start(b0_sb[:], b0r[:])
            ident_sb = constp.tile([P, 128], BF16)
            nc.sync.dma_start(ident_sb[:], ident_t[:])
            w1_sb = constp.tile([P, 2, G], BF16)
            nc.sync.dma_start(w1_sb[:, 0, :], w1te[0:P, :])
            nc.sync.dma_start(w1_sb[:, 1, :], w1te[P : 2 * P, :])
            h1_all = constp.tile([P, W_PER_CORE * HF], BF16)
            gb_all = constp.tile([P, W_PER_CORE, OUT_F], BF16)
            e1_all = constp.tile([P, W_PER_CORE, 2], F32)

            SKEW = 2          # back-stage lags front-stage by 2 windows
            pus = {}          # window -> live psum tile

            def front(w):
                he = pool.tile([P, C, HID, HEADS], BF16, tag="he")
                nc.sync.dma_start(
                    he[:], h_edge[w].rearrange("p (c d h) -> p c d h", d=HID, h=HEADS)
                )
                S_all = pool.tile([P, C, 128], F8, tag="S_all")
                nc.sync.dma_start(S_all[:], S_in[w].rearrange("p (c n) -> p c n", n=128))
                mt = pool.tile([P, C, 8], BF16, tag="mt")
                nc.sync.dma_start(
                    mt[:], meta[w].rearrange("p (c n) -> p c n", n=8)
                )

                # e = lrelu(el + er)  [P, C, 4]
                e = spool.tile([P, C, HEADS], BF16, tag="e")
                nc.gpsimd.tensor_tensor(
                    out=e[:], in0=mt[:, :, 0:4], in1=mt[:, :, 4:8],
                    op=mybir.AluOpType.add,
                )
                t = spool.tile([P, C, HEADS], BF16, tag="t")
                nc.gpsimd.tensor_scalar_mul(t[:], e[:], NEG_SLOPE)
                nc.vector.tensor_tensor(
                    out=e[:], in0=e[:], in1=t[:], op=mybir.AluOpType.max
                )
                ee = spool.tile([P, C, HEADS], BF16, tag="ee")
                nc.scalar.activation(ee[:], e[:], mybir.ActivationFunctionType.Exp)

                # msg[p, c, 0:256](d,h) = he * ee (one bcast mult at 2x)
                # msg[p, c, 256:260]    = ee      (denominator columns)
                msg = pool.tile([P, C, RW], BF16, tag="msg")
                ee4 = ee[:].rearrange("p c (o h) -> p c o h", o=1, h=HEADS)
                nc.vector.tensor_tensor(
                    out=msg[:, :, 0:HF].rearrange("p c (d h) -> p c d h", d=HID, h=HEADS),
                    in0=he[:],
                    in1=ee4.to_broadcast([P, C, HID, HEADS]),
                    op=mybir.AluOpType.mult,
                )
                nc.scalar.activation(
                    msg[:, :, HF : HF + 4], e[:],
                    mybir.ActivationFunctionType.Exp,
                )

                pu = psum.tile([P, RW], F32, tag="pu")
                for c in range(C):
                    nc.tensor.matmul(
                        pu[:], lhsT=S_all[:, c, :], rhs=msg[:, c, :],
                        start=(c == 0), stop=(c == C - 1),
                    )
                pus[w] = pu

            def back(w):
                pu = pus.pop(w)
                # rs = 1/sum(ee) per head (bf16), h1 = relu(num*rs) (+b0)
                rs_b = spool.tile([P, 1, HEADS], BF16, tag="rs_b")
                with nc.allow_low_precision(reason="1/sum(ee) applied to bf16 msg"):
                    if use_eps:
                        s_eps = spool.tile([P, HEADS], F32, tag="s_eps")
                        nc.vector.tensor_scalar_add(s_eps[:], pu[:, HF : HF + 4], 1e-38)
                        nc.vector.reciprocal(rs_b[:, 0, :], s_eps[:])
                    else:
                        nc.vector.reciprocal(rs_b[:, 0, :], pu[:, HF : HF + 4])
                h1w = h1_all[:, w * HF : (w + 1) * HF]
                nc.scalar.copy(h1w, pu[:, 0:HF])
                h1w4 = h1w.rearrange("p (d h) -> p d h", d=HID, h=HEADS)
                nc.vector.tensor_tensor(
                    out=h1w4, in0=h1w4, in1=rs_b[:].to_broadcast([P, HID, HEADS]),
                    op=mybir.AluOpType.mult,
                )
                if has_b0:
                    nc.vector.tensor_tensor(
                        out=h1w, in0=h1w, in1=b0_sb[:], op=mybir.AluOpType.add
                    )
                nc.vector.tensor_scalar_max(h1w, h1w, 0.0)

                # L1 node matmul for this window: g|el1|er1 = relu_h1 @ w1te
                pg = psum2.tile([P, G], F32, tag="pg")
                for kk in range(2):
                    pt = psum2.tile([P, 128], BF16, tag="pt")
                    nc.tensor.transpose(
                        out=pt[:],
                        in_=h1_all[:, w * HF + kk * P : w * HF + (kk + 1) * P],
                        identity=ident_sb[:],
                    )
                    h1t = spool.tile([P, 128], BF16, tag="h1t")
                    nc.scalar.copy(h1t[:], pt[:])
                    nc.tensor.matmul(
                        pg[:], lhsT=h1t[:], rhs=w1_sb[:, kk, :],
                        start=(kk == 0), stop=(kk == 1),
                    )
                nc.scalar.copy(gb_all[:, w, :], pg[:, 0:OUT_F])
                nc.vector.tensor_copy(e1_all[:, w, :], pg[:, OUT_F:G])

            for w in range(W_PER_CORE + SKEW):
                if w < W_PER_CORE:
                    front(w)
                if w >= SKEW:
                    back(w - SKEW)
            nc.sync.dma_start(gb.rearrange("(w p) f -> p w f", p=P), gb_all[:])
            nc.sync.dma_start(e1.rearrange("(w p) f -> p w f", p=P), e1_all[:])
    return nc


def build_k3(C, use_eps, has_b1):
    """L1 edge phase: y = (sum_e ee1*g[src]) / (sum_e ee1) + b1 per dst node."""
    nc = bass.Bass()
    g_edge = nc.dram_tensor("g_edge", [W_PER_CORE, P, C * OUT_F], BF16, kind="ExternalInput")
    meta1 = nc.dram_tensor("meta1", [W_PER_CORE, P, C * 2], BF16, kind="ExternalInput")
    S_in = nc.dram_tensor("S_in", [W_PER_CORE, P, C * 128], F8, kind="ExternalInput")
    b1r = nc.dram_tensor("b1r", [P, OUT_F], F32, kind="ExternalInput")
    y_out = nc.dram_tensor("y_out", [OWN, OUT_F], F32, kind="ExternalOutput")
    RW = OUT_F + 1                             # 65: msg | ee

    WPAIR = (W_PER_CORE + 1) // 2

    with tile.TileContext(nc) as tc:
        with (
            tc.tile_pool(name="const", bufs=1) as constp,
            tc.tile_pool(name="sbuf", bufs=4) as pool,
            tc.tile_pool(name="small", bufs=4) as spool,
            tc.tile_pool(name="psum", bufs=4, space="PSUM") as psum,
        ):
            b1_sb = constp.tile([P, OUT_F], F32)
            nc.sync.dma_start(b1_sb[:], b1r[:])
            y_all = constp.tile([P, W_PER_CORE, OUT_F], F32)

            SKEW = 2
            pus = {}
            tiles = {}

            def front(w):
                if w % 2 == 0:
                    nw = 2 if w + 1 < W_PER_CORE else 1
                    ge = pool.tile([P, 2, C, OUT_F], BF16, tag="ge")
                    nc.sync.dma_start(
                        ge[:, 0:nw],
                        g_edge[w : w + nw].rearrange("w p (c f) -> p w c f", f=OUT_F),
                    )
                    S_all = pool.tile([P, 2, C, 128], F8, tag="S_all")
                    nc.sync.dma_start(
                        S_all[:, 0:nw],
                        S_in[w : w + nw].rearrange("w p (c n) -> p w c n", n=128),
                    )
                    mt = pool.tile([P, 2, C, 2], BF16, tag="mt")
                    nc.scalar.dma_start(
                        mt[:, 0:nw],
                        meta1[w : w + nw].rearrange("w p (c n) -> p w c n", n=2),
                    )
                    tiles[w // 2] = (ge, S_all, mt)
                ge, S_all, mt = tiles[w // 2]
                wi = w % 2
                # e = lrelu(el + er), ee = exp(e)
                e = spool.tile([P, C, 1], BF16, tag="e")
                nc.gpsimd.tensor_tensor(
                    out=e[:], in0=mt[:, wi, :, 0:1], in1=mt[:, wi, :, 1:2],
                    op=mybir.AluOpType.add,
                )
                t = spool.tile([P, C, 1], BF16, tag="t")
                nc.vector.tensor_scalar_mul(t[:], e[:], NEG_SLOPE)
                nc.vector.tensor_tensor(
                    out=e[:], in0=e[:], in1=t[:], op=mybir.AluOpType.max
                )
                ee = spool.tile([P, C, 1], BF16, tag="ee")
                nc.scalar.activation(ee[:], e[:], mybir.ActivationFunctionType.Exp)

                msg = pool.tile([P, C, RW], BF16, tag="msg")
                nc.vector.tensor_tensor(
                    out=msg[:, :, 0:OUT_F], in0=ge[:, wi],
                    in1=ee[:].to_broadcast([P, C, OUT_F]),
                    op=mybir.AluOpType.mult,
                )
                nc.scalar.activation(
                    msg[:, :, OUT_F : OUT_F + 1], e[:],
                    mybir.ActivationFunctionType.Exp,
                )

                pu = psum.tile([P, RW], F32, tag="pu")
                for c in range(C):
                    nc.tensor.matmul(
                        pu[:], lhsT=S_all[:, wi, c, :], rhs=msg[:, c, :],
                        start=(c == 0), stop=(c == C - 1),
                    )
                pus[w] = pu

            def back(w):
                pu = pus.pop(w)
                rs = spool.tile([P, 1], F32, tag="rs")
                if use_eps:
                    s_eps = spool.tile([P, 1], F32, tag="s_eps")
                    nc.vector.tensor_scalar_add(
                        s_eps[:], pu[:, OUT_F : OUT_F + 1], 1e-38
                    )
                    nc.vector.reciprocal(rs[:], s_eps[:])
                else:
                    nc.vector.reciprocal(rs[:], pu[:, OUT_F : OUT_F + 1])
                yw = y_all[:, w, :]
                # y = num * rs (+ b1): scalar Copy activation with scale
                nc.scalar.activation(
                    yw, pu[:, 0:OUT_F],
                    mybir.ActivationFunctionType.Copy, scale=rs[:, 0:1],
                )
                if has_b1:
                    nc.vector.tensor_tensor(
                        out=yw, in0=yw, in1=b1_sb[:], op=mybir.AluOpType.add
                    )

            for w in range(W_PER_CORE + SKEW):
                if w < W_PER_CORE:
                    front(w)
                if w >= SKEW:
                    back(w - SKEW)
            nc.sync.dma_start(y_out.rearrange("(w p) f -> p w f", p=P), y_all[:])
    return nc


# ------------------------------------------------------------- host helpers
def _run(nc, in_maps, label):
    profile = os.environ.get("GAT_PROFILE", "0") == "1"
    res = run_bass_kernel_spmd(
        nc, in_maps, core_ids=list(range(NC_CORES)), trace=profile
    )
    if profile:
        EXEC_TIMES_NS[label] = res.exec_time_ns
    return res.results


def _edge_slots(src, dst):
    """Per-core edge->slot assignment.  Returns (C, sidx, ddst, dloc, min_deg)."""
    core = dst // OWN
    win = (dst - core * OWN) // P
    loc = (dst - core * OWN) % P

    counts = np.zeros((NC_CORES, W_PER_CORE), dtype=np.int64)
    np.add.at(counts, (core, win), 1)
    C = int(np.ceil(counts.max() / P))

    deg = np.bincount(dst, minlength=PADN)[:N_NODES]
    min_deg = int(deg.min())

    order = np.lexsort((win, core))
    s_src, s_core, s_win, s_loc = src[order], core[order], win[order], loc[order]
    group = s_core * W_PER_CORE + s_win
    gstart = np.zeros(NC_CORES * W_PER_CORE, dtype=np.int64)
    cnt = np.bincount(group, minlength=NC_CORES * W_PER_CORE)
    gstart[1:] = np.cumsum(cnt)[:-1]
    within = np.arange(len(order)) - gstart[group]

    sidx = np.full((NC_CORES, W_PER_CORE, C * P), -1, dtype=np.int64)
    ddst = np.full((NC_CORES, W_PER_CORE, C * P), -1, dtype=np.int64)
    dloc = np.full((NC_CORES, W_PER_CORE, C * P), -1.0, dtype=np.float32)
    sidx[s_core, s_win, within] = s_src
    ddst[s_core, s_win, within] = s_core * OWN + s_win * P + s_loc
    dloc[s_core, s_win, within] = s_loc.astype(np.float32)
    return C, sidx, ddst, dloc, min_deg


def _to_tiles(rows, C, ncol):
    """[W, C*P, ncol] -> [W, P, C*ncol] (slot j -> partition j%P, chunk j//P)."""
    W = rows.shape[0]
    return (
        rows.reshape(W, C, P, ncol).transpose(0, 2, 1, 3).reshape(W, P, C * ncol)
    )


def kernel(x, src, dst, W0, al0, ar0, b0, W1, al1, ar1, b1):
    _patch_tile()
    _install_ntff_hook()

    x = np.asarray(x, dtype=np.float32)
    src = np.asarray(src, dtype=np.int64)
    dst = np.asarray(dst, dtype=np.int64)
    W0 = np.asarray(W0, dtype=np.float32)
    al0 = np.asarray(al0, dtype=np.float32)
    ar0 = np.asarray(ar0, dtype=np.float32)
    b0 = np.asarray(b0, dtype=np.float32)
    W1 = np.asarray(W1, dtype=np.float32)
    al1 = np.asarray(al1, dtype=np.float32)
    ar1 = np.asarray(ar1, dtype=np.float32)
    b1 = np.asarray(b1, dtype=np.float32)

    HF = HEADS * HID
    G = OUT_F + 2

    import ml_dtypes

    BF = ml_dtypes.bfloat16
    F8H = ml_dtypes.float8_e4m3

    # (d,h)-interleave permutation over the 256 hidden columns
    il = (np.arange(HF).reshape(HEADS, HID).T).reshape(-1)   # il[d*4+h] = h*64+d

    # ---- weight prep
    vl0 = np.einsum("hd,hdk->hk", al0, W0.reshape(HEADS, HID, IN_F))   # [4, 256]
    vr0 = np.einsum("hd,hdk->hk", ar0, W0.reshape(HEADS, HID, IN_F))
    w0te = np.concatenate([W0.T, vl0.T, vr0.T], axis=1).astype(BF)     # [256, 264]
    vl1 = al1 @ W1                                                      # [1, 256]
    vr1 = ar1 @ W1
    w1te = np.concatenate([W1.T, vl1.T, vr1.T], axis=1)[il].astype(BF)  # [256, 66] permuted rows

    xT_pad = np.zeros((IN_F, PADN), dtype=BF)
    xT_pad[:, :N_NODES] = x.T.astype(BF)

    ident = np.eye(128, dtype=BF)
    b0r = np.tile(b0[il][None, :], (P, 1)).astype(BF)
    b1r = np.tile(b1[None, :], (P, 1)).astype(np.float32)
    has_b0 = bool(np.any(b0))
    has_b1 = bool(np.any(b1))

    # ---- K1: node tables
    nc1 = build_k1()
    in1 = [
        {"xT_own": np.ascontiguousarray(xT_pad[:, k * OWN : (k + 1) * OWN]), "w0te": w0te}
        for k in range(NC_CORES)
    ]
    r1 = _run(nc1, in1, "k1")
    hbtab = np.concatenate([r1[k]["hb"] for k in range(NC_CORES)], axis=0)   # [PADN,256] bf16
    elrtab = np.concatenate([r1[k]["elr"] for k in range(NC_CORES)], axis=0)  # [PADN,8] f32

    # ---- edge layout
    C, sidx, ddst, dloc, min_deg = _edge_slots(src, dst)
    use_eps = min_deg == 0

    # pre-permute table columns to (d,h)-interleaved order once
    hbtab_il = np.ascontiguousarray(hbtab[:, il])
    hbtab_x = np.concatenate([hbtab_il, np.zeros((1, IN_F), dtype=BF)], axis=0)
    elrtab_x = np.concatenate([elrtab, np.zeros((1, 8), dtype=np.float32)], axis=0)

    s_cl = np.where(sidx < 0, PADN, sidx)
    d_cl = np.where(ddst < 0, PADN, ddst)

    # one-hot tiles: S[w, p, c*128+n] = (dstloc == n)
    def s_tiles(dl):
        oh = (dl.reshape(W_PER_CORE, C, P)[:, :, :, None]
              == np.arange(128, dtype=np.float32)[None, None, None, :])
        return np.ascontiguousarray(
            oh.transpose(0, 2, 1, 3).reshape(W_PER_CORE, P, C * 128).astype(F8H)
        )

    # ---- K2 inputs
    nc2 = build_k2(C, use_eps, has_b0)
    in2 = []
    for k in range(NC_CORES):
        h_edge = _to_tiles(hbtab_x[s_cl[k], :], C, HF)          # bf16 byte gather
        meta = np.empty((W_PER_CORE, C * P, 8), dtype=np.float32)
        meta[:, :, 0:4] = elrtab_x[s_cl[k], 0:4]
        meta[:, :, 4:8] = elrtab_x[d_cl[k], 4:8]
        meta = _to_tiles(meta, C, 8).astype(BF)
        in2.append(
            {
                "h_edge": np.ascontiguousarray(h_edge),
                "meta": np.ascontiguousarray(meta),
                "S_in": s_tiles(dloc[k]),
                "b0r": b0r,
                "ident": ident,
                "w1te": w1te,
            }
        )
    r2 = _run(nc2, in2, "k2")
    gbtab = np.concatenate([r2[k]["gb"] for k in range(NC_CORES)], axis=0)   # [PADN,64] bf16
    e1tab = np.concatenate([r2[k]["e1"] for k in range(NC_CORES)], axis=0)   # [PADN,2] f32
    gbtab_x = np.concatenate([gbtab, np.zeros((1, OUT_F), dtype=BF)], axis=0)
    e1tab_x = np.concatenate([e1tab, np.zeros((1, 2), dtype=np.float32)], axis=0)

    # ---- K3 inputs
    nc3 = build_k3(C, use_eps, has_b1)
    in3 = []
    for k in range(NC_CORES):
        g_edge = _to_tiles(gbtab_x[s_cl[k], :], C, OUT_F)       # bf16 byte gather
        meta1 = np.empty((W_PER_CORE, C * P, 2), dtype=np.float32)
        meta1[:, :, 0] = e1tab_x[s_cl[k], 0]
        meta1[:, :, 1] = e1tab_x[d_cl[k], 1]
        meta1 = _to_tiles(meta1, C, 2).astype(BF)
        in3.append(
            {
                "g_edge": np.ascontiguousarray(g_edge),
                "meta1": np.ascontiguousarray(meta1),
                "S_in": in2[k]["S_in"],
                "b1r": b1r,
            }
        )
    r3 = _run(nc3, in3, "k3")
    y = np.concatenate([r3[k]["y_out"] for k in range(NC_CORES)], axis=0)
    return np.ascontiguousarray(y[:N_NODES]).astype(np.float32)


# revision 46
# speedup vs baseline: 1.2465x; 1.0360x over previous
"""Two-layer GAT (4-head then 1-head) on 8 NeuronCores.

Sharding: nodes are partitioned across the 8 cores by dst-ownership
(6272 = 49*128 aligned nodes per core).  Each core processes all edges whose
dst it owns.  Per-dst-window (128 nodes) the segment softmax + weighted
aggregation run as one-hot-selection matmuls on the tensor engine.

Edge-gathered features are stored d-major ([64][4heads] interleaved) so the
per-edge softmax weight applies as ONE stride-0-broadcast DVE multiply at
2-byte speed (no broadcast materialization, no big scalar-engine pass).

Three SPMD launches:
  K1: h|el|er = x @ [W0^T | vl0^T | vr0^T]   (node-sharded, bf16)
  K2: L0 edge phase (attention + aggregation) + relu + g|el1|er1 matmul
  K3: L1 edge phase -> output

Between launches the host performs pure index gathers / dtype casts of
device-computed tables; all floating-point math runs on device.
"""
import os
import sys
import types

sys.path.insert(0, "/opt/trn_rl_repo")

import numpy as np

import concourse.bass as bass
import concourse.tile as tile
from concourse import mybir
from concourse.bass_utils import run_bass_kernel_spmd
from concourse.vector_clock import ScopedClock

# ---------------------------------------------------------------- constants
N_NODES = int(os.environ.get("GAT_N_NODES", "50000"))
IN_F = 256
HID = 64
HEADS = 4
OUT_F = 64
NEG_SLOPE = 0.2

NC_CORES = 8
P = 128
W_PER_CORE = int(os.environ.get("GAT_W", "49"))
OWN = W_PER_CORE * P            # 6272 nodes per core
PADN = NC_CORES * OWN           # 50176
F32 = mybir.dt.float32
BF16 = mybir.dt.bfloat16
F8 = mybir.dt.float8e4

EXEC_TIMES_NS = {}              # filled when GAT_PROFILE=1


# ------------------------------------------------------------- tile patches
def _patch_tile():
    """This container's walrus rejects instructions with >1 sem wait
    ("Too many sync wait commands").  After Tile lowering, move excess waits
    onto same-engine no-ops inserted before the offending instruction."""
    if getattr(_patch_tile, "done", False):
        return
    _patch_tile.done = True

    MAX_WAITS = 1

    def _split_all_waits(nc):
        for bb in nc.main_func.blocks:
            insts = bb.instructions
            i = 0
            while i < len(insts):
                inst = insts[i]
                si = getattr(inst, "sync_info", None)
                if si is None or len(si.on_wait) <= MAX_WAITS:
                    i += 1
                    continue
                waits = list(si.on_wait)
                si.on_wait[:] = waits[:MAX_WAITS]
                extra = waits[MAX_WAITS:]
                nops = []
                for j in range(0, len(extra), MAX_WAITS):
                    nop = mybir.InstNoOp(
                        name=f"I-waitsplit-{nc.next_id()}",
                        ins=[],
                        outs=[],
                        engine=inst.engine,
                    )
                    nop.sync_info = mybir.SyncInfo(
                        on_wait=extra[j : j + MAX_WAITS], on_update=[]
                    )
                    nc.register_instruction(nop, overwrite=True)
                    nops.append(nop)
                insts[i:i] = nops
                i += len(nops) + 1

    def _drain_and_barrier(self, tick_clock, wait_clock):
        drain_inst = self.nc.sync.drain()
        wait_clock.add_sem_waits(
            drain_inst.ins, ScopedClock({None: tick_clock.global_clock})
        )
        self.nc.all_engine_barrier()
        assert self.sems is not None
        popped = self.nc._tile_sem_poison_stack.pop()
        assert popped is self._sem_poison
        self.nc.clear_and_free_semaphores(list(self.sems.allocated().values()))
        self.nc.all_engine_barrier()
        _split_all_waits(self.nc)

    tile.TileContext._drain_and_barrier = _drain_and_barrier


def _install_ntff_hook():
    """Enable run_bass_kernel_spmd(trace=True) under axon: register the NTFF
    profile hook that the boot script skips when antenv.axon_hooks is absent."""
    if getattr(_install_ntff_hook, "done", False):
        return
    _install_ntff_hook.done = True
    try:
        mod = types.ModuleType("antenv.axon_hooks")
        _state = {}

        def set_axon_ntff_profile_hook(h):
            _state["h"] = h

        def get_axon_ntff_profile_hook():
            return _state.get("h")

        mod.set_axon_ntff_profile_hook = set_axon_ntff_profile_hook
        mod.get_axon_ntff_profile_hook = get_axon_ntff_profile_hook
        sys.modules["antenv.axon_hooks"] = mod
        import antenv

        antenv.axon_hooks = mod
        from trn_agent_boot.trn_boot import _ntff_profile_via_ctypes

        hook = _ntff_profile_via_ctypes("/opt/axon/libaxon_pjrt.so")
        if hook is not None:
            set_axon_ntff_profile_hook(hook)
    except Exception:
        pass


# ------------------------------------------------------------- kernel builders
def build_k1():
    """h (bf16) | el,er (f32) tables for this core's 6272 nodes.

    Streams x in 7-window chunks and the output tables back out per chunk so
    DMA, PE, and the psum-evacuation copies overlap instead of serializing.
    """
    nc = bass.Bass()
    DE = IN_F + 2 * HEADS                     # 264
    CH = 7                                    # windows per chunk
    NCH = W_PER_CORE // CH                    # 7 chunks
    assert CH * NCH == W_PER_CORE
    xT_own = nc.dram_tensor("xT_own", [IN_F, OWN], BF16, kind="ExternalInput")
    w0te = nc.dram_tensor("w0te", [IN_F, DE], BF16, kind="ExternalInput")
    hb = nc.dram_tensor("hb", [OWN, IN_F], BF16, kind="ExternalOutput")
    elr = nc.dram_tensor("elr", [OWN, 2 * HEADS], F32, kind="ExternalOutput")

    with tile.TileContext(nc) as tc:
        with (
            tc.tile_pool(name="const", bufs=1) as constp,
            tc.tile_pool(name="sbuf", bufs=3) as pool,
            tc.tile_pool(name="psum", bufs=4, space="PSUM") as psum,
        ):
            wt = constp.tile([P, 2, DE], BF16)
            nc.sync.dma_start(wt[:, 0, :], w0te[0:P, :])
            nc.sync.dma_start(wt[:, 1, :], w0te[P : 2 * P, :])
            for ci in range(NCH):
                lo = ci * CH * P
                xkc = pool.tile([P, 2, CH * P], BF16, tag="xkc")
                nc.sync.dma_start(xkc[:, 0, :], xT_own[0:P, lo : lo + CH * P])
                nc.sync.dma_start(xkc[:, 1, :], xT_own[P : 2 * P, lo : lo + CH * P])
                hbc = pool.tile([P, CH, IN_F], BF16, tag="hbc")
                elrc = pool.tile([P, CH, 2 * HEADS], F32, tag="elrc")
                for mi in range(CH):
                    pu = psum.tile([P, DE], F32, tag="pu")
                    for kk in range(2):
                        nc.tensor.matmul(
                            pu[:],
                            lhsT=xkc[:, kk, mi * P : (mi + 1) * P],
                            rhs=wt[:, kk, :],
                            start=(kk == 0),
                            stop=(kk == 1),
                        )
                    nc.scalar.copy(hbc[:, mi, :], pu[:, 0:IN_F])
                    nc.vector.tensor_copy(elrc[:, mi, :], pu[:, IN_F:DE])
                hbd = hb.rearrange("(w p) f -> p w f", p=P)
                elrd = elr.rearrange("(w p) f -> p w f", p=P)
                nc.sync.dma_start(hbd[:, ci * CH : (ci + 1) * CH, :], hbc[:])
                nc.scalar.dma_start(elrd[:, ci * CH : (ci + 1) * CH, :], elrc[:])
    return nc


def build_k2(C, use_eps, has_b0):
    """L0 edge phase + relu + L1 node matmul.

    Inputs (per core):
      h_edge [W, P, C*256] bf16  gathered h rows (src), d-major (d,h) interleave
      meta   [W, P, C*8]   bf16  per chunk: el(4) | er(4)
      S_in   [W, P, C*128] fp8   one-hot dst-lane selectors
      b0r    [P, 256]      bf16  (d,h)-interleaved b0 rows
      ident  [P, 128]      bf16
      w1te   [256, 66]     bf16  rows (d,h)-interleaved
    Outputs:
      gb  [OWN, 64] bf16   g rows for this core's nodes
      e1  [OWN, 2]  f32    el1 | er1
    """
    nc = bass.Bass()
    HF = HEADS * HID                           # 256
    G = OUT_F + 2                              # 66
    RW = HF + 4                                # 260 msg row width
    h_edge = nc.dram_tensor("h_edge", [W_PER_CORE, P, C * HF], BF16, kind="ExternalInput")
    meta = nc.dram_tensor("meta", [W_PER_CORE, P, C * 8], BF16, kind="ExternalInput")
    S_in = nc.dram_tensor("S_in", [W_PER_CORE, P, C * 128], F8, kind="ExternalInput")
    b0r = nc.dram_tensor("b0r", [P, HF], BF16, kind="ExternalInput")
    ident_t = nc.dram_tensor("ident", [P, 128], BF16, kind="ExternalInput")
    w1te = nc.dram_tensor("w1te", [HF, G], BF16, kind="ExternalInput")
    gb = nc.dram_tensor("gb", [OWN, OUT_F], BF16, kind="ExternalOutput")
    e1 = nc.dram_tensor("e1", [OWN, 2], F32, kind="ExternalOutput")

    with tile.TileContext(nc) as tc:
        with (
            tc.tile_pool(name="const", bufs=1) as constp,
            tc.tile_pool(name="sbuf", bufs=4) as pool,
            tc.tile_pool(name="small", bufs=4) as spool,
            tc.tile_pool(name="psum", bufs=4, space="PSUM") as psum,
            tc.tile_pool(name="psum2", bufs=2, space="PSUM") as psum2,
        ):
            b0_sb = constp.tile([P, HF], BF16)
            nc.sync.dma_start(b0_sb[:], b0r[:])
            ident_sb = constp.tile([P, 128], BF16)
            nc.sync.dma_start(ident_sb[:], ident_t[:])
            w1_sb = constp.tile([P, 2, G], BF16)
            nc.sync.dma_start(w1_sb[:, 0, :], w1te[0:P, :])
            nc.sync.dma_start(w1_sb[:, 1, :], w1te[P : 2 * P, :])
            h1_all = constp.tile([P, W_PER_CORE * HF], BF16)
            gb_all = constp.tile([P, W_PER_CORE, OUT_F], BF16)
            e1_all = constp.tile([P, W_PER_CORE, 2], F32)

            SKEW = 2          # back-stage lags the compute stage by 2 windows
            PF = 2            # DMA prefetch distance (windows)
            EPF = 1           # e-pipeline lookahead
            pus = {}          # window -> live psum tile
            dmas = {}         # window -> (he, S_all, mt)
            ees = {}          # window -> (e, ee)

            def fetch(w):
                he = pool.tile([P, C, HID, HEADS], BF16, tag="he")
                nc.sync.dma_start(
                    he[:], h_edge[w].rearrange("p (c d h) -> p c d h", d=HID, h=HEADS)
                )
                S_all = pool.tile([P, C, 128], F8, tag="S_all")
                nc.sync.dma_start(S_all[:], S_in[w].rearrange("p (c n) -> p c n", n=128))
                mt = pool.tile([P, C, 8], BF16, tag="mt")
                nc.scalar.dma_start(
                    mt[:], meta[w].rearrange("p (c n) -> p c n", n=8)
                )
                dmas[w] = (he, S_all, mt)

            def epipe(w):
                mt = dmas[w][2]
                # e = lrelu(el + er)  [P, C, 4]
                e = spool.tile([P, C, HEADS], BF16, tag="e")
                nc.gpsimd.tensor_tensor(
                    out=e[:], in0=mt[:, :, 0:4], in1=mt[:, :, 4:8],
                    op=mybir.AluOpType.add,
                )
                t = spool.tile([P, C, HEADS], BF16, tag="t")
                nc.gpsimd.tensor_scalar_mul(t[:], e[:], NEG_SLOPE)
                nc.vector.tensor_tensor(
                    out=e[:], in0=e[:], in1=t[:], op=mybir.AluOpType.max
                )
                ee = spool.tile([P, C, HEADS], BF16, tag="ee")
                nc.scalar.activation(ee[:], e[:], mybir.ActivationFunctionType.Exp)
                ees[w] = (e, ee)

            def front(w):
                he, S_all, _ = dmas.pop(w)
                e, ee = ees.pop(w)
                # msg[p, c, 0:256](d,h) = he * ee (one bcast mult at 2x)
                # msg[p, c, 256:260]    = ee      (denominator columns)
                msg = pool.tile([P, C, RW], BF16, tag="msg")
                ee4 = ee[:].rearrange("p c (o h) -> p c o h", o=1, h=HEADS)
                nc.vector.tensor_tensor(
                    out=msg[:, :, 0:HF].rearrange("p c (d h) -> p c d h", d=HID, h=HEADS),
                    in0=he[:],
                    in1=ee4.to_broadcast([P, C, HID, HEADS]),
                    op=mybir.AluOpType.mult,
                )
                nc.scalar.activation(
                    msg[:, :, HF : HF + 4], e[:],
                    mybir.ActivationFunctionType.Exp,
                )

                pu = psum.tile([P, RW], F32, tag="pu")
                for c in range(C):
                    nc.tensor.matmul(
                        pu[:], lhsT=S_all[:, c, :], rhs=msg[:, c, :],
                        start=(c == 0), stop=(c == C - 1),
                    )
                pus[w] = pu

            def back(w):
                pu = pus.pop(w)
                # rs = 1/sum(ee) per head (bf16), h1 = relu(num*rs) (+b0)
                rs_b = spool.tile([P, 1, HEADS], BF16, tag="rs_b")
                with nc.allow_low_precision(reason="1/sum(ee) applied to bf16 msg"):
                    if use_eps:
                        s_eps = spool.tile([P, HEADS], F32, tag="s_eps")
                        nc.vector.tensor_scalar_add(s_eps[:], pu[:, HF : HF + 4], 1e-38)
                        nc.vector.reciprocal(rs_b[:, 0, :], s_eps[:])
                    else:
                        nc.vector.reciprocal(rs_b[:, 0, :], pu[:, HF : HF + 4])
                h1w = h1_all[:, w * HF : (w + 1) * HF]
                nc.scalar.copy(h1w, pu[:, 0:HF])
                h1w4 = h1w.rearrange("p (d h) -> p d h", d=HID, h=HEADS)
                nc.vector.tensor_tensor(
                    out=h1w4, in0=h1w4, in1=rs_b[:].to_broadcast([P, HID, HEADS]),
                    op=mybir.AluOpType.mult,
                )
                if has_b0:
                    nc.vector.tensor_tensor(
                        out=h1w, in0=h1w, in1=b0_sb[:], op=mybir.AluOpType.add
                    )
                nc.vector.tensor_scalar_max(h1w, h1w, 0.0)

                # L1 node matmul for this window: g|el1|er1 = relu_h1 @ w1te
                pg = psum2.tile([P, G], F32, tag="pg")
                for kk in range(2):
                    pt = psum2.tile([P, 128], BF16, tag="pt")
                    nc.tensor.transpose(
                        out=pt[:],
                        in_=h1_all[:, w * HF + kk * P : w * HF + (kk + 1) * P],
                        identity=ident_sb[:],
                    )
                    h1t = spool.tile([P, 128], BF16, tag="h1t")
                    nc.scalar.copy(h1t[:], pt[:])
                    nc.tensor.matmul(
                        pg[:], lhsT=h1t[:], rhs=w1_sb[:, kk, :],
                        start=(kk == 0), stop=(kk == 1),
                    )
                nc.scalar.copy(gb_all[:, w, :], pg[:, 0:OUT_F])
                nc.vector.tensor_copy(e1_all[:, w, :], pg[:, OUT_F:G])

            for w in range(W_PER_CORE + SKEW):
                if w == 0:
                    for pw in range(min(PF + 1, W_PER_CORE)):
                        fetch(pw)
                    for pw in range(min(EPF + 1, W_PER_CORE)):
                        epipe(pw)
                else:
                    if w + PF < W_PER_CORE:
                        fetch(w + PF)
                    if w + EPF < W_PER_CORE:
                        epipe(w + EPF)
                if w < W_PER_CORE:
                    front(w)
                if w >= SKEW:
                    back(w - SKEW)
            nc.sync.dma_start(gb.rearrange("(w p) f -> p w f", p=P), gb_all[:])
            nc.sync.dma_start(e1.rearrange("(w p) f -> p w f", p=P), e1_all[:])
    return nc


def build_k3(C, use_eps, has_b1):
    """L1 edge phase: y = (sum_e ee1*g[src]) / (sum_e ee1) + b1 per dst node."""
    nc = bass.Bass()
    g_edge = nc.dram_tensor("g_edge", [W_PER_CORE, P, C * OUT_F], BF16, kind="ExternalInput")
    meta1 = nc.dram_tensor("meta1", [W_PER_CORE, P, C * 2], BF16, kind="ExternalInput")
    S_in = nc.dram_tensor("S_in", [W_PER_CORE, P, C * 128], F8, kind="ExternalInput")
    b1r = nc.dram_tensor("b1r", [P, OUT_F], F32, kind="ExternalInput")
    y_out = nc.dram_tensor("y_out", [OWN, OUT_F], F32, kind="ExternalOutput")
    RW = OUT_F + 1                             # 65: msg | ee

    WPAIR = (W_PER_CORE + 1) // 2

    with tile.TileContext(nc) as tc:
        with (
            tc.tile_pool(name="const", bufs=1) as constp,
            tc.tile_pool(name="sbuf", bufs=4) as pool,
            tc.tile_pool(name="small", bufs=4) as spool,
            tc.tile_pool(name="psum", bufs=4, space="PSUM") as psum,
        ):
            b1_sb = constp.tile([P, OUT_F], F32)
            nc.sync.dma_start(b1_sb[:], b1r[:])
            y_all = constp.tile([P, W_PER_CORE, OUT_F], F32)

            SKEW = 2
            pus = {}
            tiles = {}

            def fetch_pair(pi):
                w0 = 2 * pi
                if w0 >= W_PER_CORE:
                    return
                nw = 2 if w0 + 1 < W_PER_CORE else 1
                ge = pool.tile([P, 2, C, OUT_F], BF16, tag="ge")
                nc.sync.dma_start(
                    ge[:, 0:nw],
                    g_edge[w0 : w0 + nw].rearrange("w p (c f) -> p w c f", f=OUT_F),
                )
                S_all = pool.tile([P, 2, C, 128], F8, tag="S_all")
                nc.sync.dma_start(
                    S_all[:, 0:nw],
                    S_in[w0 : w0 + nw].rearrange("w p (c n) -> p w c n", n=128),
                )
                mt = pool.tile([P, 2, C, 2], BF16, tag="mt")
                nc.scalar.dma_start(
                    mt[:, 0:nw],
                    meta1[w0 : w0 + nw].rearrange("w p (c n) -> p w c n", n=2),
                )
                tiles[pi] = (ge, S_all, mt)

            def front(w):
                if w == 0:
                    fetch_pair(0)
                    fetch_pair(1)
                if w % 2 == 0:
                    fetch_pair(w // 2 + 2)
                ge, S_all, mt = tiles[w // 2]
                wi = w % 2
                # e = lrelu(el + er), ee = exp(e)
                e = spool.tile([P, C, 1], BF16, tag="e")
                nc.gpsimd.tensor_tensor(
                    out=e[:], in0=mt[:, wi, :, 0:1], in1=mt[:, wi, :, 1:2],
                    op=mybir.AluOpType.add,
                )
                t = spool.tile([P, C, 1], BF16, tag="t")
                nc.vector.tensor_scalar_mul(t[:], e[:], NEG_SLOPE)
                nc.vector.tensor_tensor(
                    out=e[:], in0=e[:], in1=t[:], op=mybir.AluOpType.max
                )
                ee = spool.tile([P, C, 1], BF16, tag="ee")
                nc.scalar.activation(ee[:], e[:], mybir.ActivationFunctionType.Exp)

                msg = pool.tile([P, C, RW], BF16, tag="msg")
                nc.vector.tensor_tensor(
                    out=msg[:, :, 0:OUT_F], in0=ge[:, wi],
                    in1=ee[:].to_broadcast([P, C, OUT_F]),
                    op=mybir.AluOpType.mult,
                )
                nc.scalar.activation(
                    msg[:, :, OUT_F : OUT_F + 1], e[:],
                    mybir.ActivationFunctionType.Exp,
                )

                pu = psum.tile([P, RW], F32, tag="pu")
                for c in range(C):
                    nc.tensor.matmul(
                        pu[:], lhsT=S_all[:, wi, c, :], rhs=msg[:, c, :],
                        start=(c == 0), stop=(c == C - 1),
                    )
                pus[w] = pu

            def back(w):
                pu = pus.pop(w)
                rs = spool.tile([P, 1], F32, tag="rs")
                if use_eps:
                    s_eps = spool.tile([P, 1], F32, tag="s_eps")
                    nc.vector.tensor_scalar_add(
                        s_eps[:], pu[:, OUT_F : OUT_F + 1], 1e-38
                    )
                    nc.vector.reciprocal(rs[:], s_eps[:])
                else:
                    nc.vector.reciprocal(rs[:], pu[:, OUT_F : OUT_F + 1])
                yw = y_all[:, w, :]
                # y = num * rs (+ b1): scalar Copy activation with scale
                nc.scalar.activation(
                    yw, pu[:, 0:OUT_F],
                    mybir.ActivationFunctionType.Copy, scale=rs[:, 0:1],
                )
                if has_b1:
                    nc.vector.tensor_tensor(
                        out=yw, in0=yw, in1=b1_sb[:], op=mybir.AluOpType.add
                    )

            for w in range(W_PER_CORE + SKEW):
                if w < W_PER_CORE:
                    front(w)
                if w >= SKEW:
                    back(w - SKEW)
            nc.sync.dma_start(y_out.rearrange("(w p) f -> p w f", p=P), y_all[:])
    return nc


# ------------------------------------------------------------- host helpers
def _run(nc, in_maps, label):
    profile = os.environ.get("GAT_PROFILE", "0") == "1"
    res = run_bass_kernel_spmd(
        nc, in_maps, core_ids=list(range(NC_CORES)), trace=profile
    )
    if profile:
        EXEC_TIMES_NS[label] = res.exec_time_ns
    return res.results


def _edge_slots(src, dst):
    """Per-core edge->slot assignment.  Returns (C, sidx, ddst, dloc, min_deg)."""
    core = dst // OWN
    win = (dst - core * OWN) // P
    loc = (dst - core * OWN) % P

    counts = np.zeros((NC_CORES, W_PER_CORE), dtype=np.int64)
    np.add.at(counts, (core, win), 1)
    C = int(np.ceil(counts.max() / P))

    deg = np.bincount(dst, minlength=PADN)[:N_NODES]
    min_deg = int(deg.min())

    order = np.lexsort((win, core))
    s_src, s_core, s_win, s_loc = src[order], core[order], win[order], loc[order]
    group = s_core * W_PER_CORE + s_win
    gstart = np.zeros(NC_CORES * W_PER_CORE, dtype=np.int64)
    cnt = np.bincount(group, minlength=NC_CORES * W_PER_CORE)
    gstart[1:] = np.cumsum(cnt)[:-1]
    within = np.arange(len(order)) - gstart[group]

    sidx = np.full((NC_CORES, W_PER_CORE, C * P), -1, dtype=np.int64)
    ddst = np.full((NC_CORES, W_PER_CORE, C * P), -1, dtype=np.int64)
    dloc = np.full((NC_CORES, W_PER_CORE, C * P), -1.0, dtype=np.float32)
    sidx[s_core, s_win, within] = s_src
    ddst[s_core, s_win, within] = s_core * OWN + s_win * P + s_loc
    dloc[s_core, s_win, within] = s_loc.astype(np.float32)
    return C, sidx, ddst, dloc, min_deg


def _to_tiles(rows, C, ncol):
    """[W, C*P, ncol] -> [W, P, C*ncol] (slot j -> partition j%P, chunk j//P)."""
    W = rows.shape[0]
    return (
        rows.reshape(W, C, P, ncol).transpose(0, 2, 1, 3).reshape(W, P, C * ncol)
    )


def kernel(x, src, dst, W0, al0, ar0, b0, W1, al1, ar1, b1):
    _patch_tile()
    _install_ntff_hook()

    x = np.asarray(x, dtype=np.float32)
    src = np.asarray(src, dtype=np.int64)
    dst = np.asarray(dst, dtype=np.int64)
    W0 = np.asarray(W0, dtype=np.float32)
    al0 = np.asarray(al0, dtype=np.float32)
    ar0 = np.asarray(ar0, dtype=np.float32)
    b0 = np.asarray(b0, dtype=np.float32)
    W1 = np.asarray(W1, dtype=np.float32)
    al1 = np.asarray(al1, dtype=np.float32)
    ar1 = np.asarray(ar1, dtype=np.float32)
    b1 = np.asarray(b1, dtype=np.float32)

    HF = HEADS * HID
    G = OUT_F + 2

    import ml_dtypes

    BF = ml_dtypes.bfloat16
    F8H = ml_dtypes.float8_e4m3

    # (d,h)-interleave permutation over the 256 hidden columns
    il = (np.arange(HF).reshape(HEADS, HID).T).reshape(-1)   # il[d*4+h] = h*64+d

    # ---- weight prep
    vl0 = np.einsum("hd,hdk->hk", al0, W0.reshape(HEADS, HID, IN_F))   # [4, 256]
    vr0 = np.einsum("hd,hdk->hk", ar0, W0.reshape(HEADS, HID, IN_F))
    w0te = np.concatenate([W0.T, vl0.T, vr0.T], axis=1).astype(BF)     # [256, 264]
    vl1 = al1 @ W1                                                      # [1, 256]
    vr1 = ar1 @ W1
    w1te = np.concatenate([W1.T, vl1.T, vr1.T], axis=1)[il].astype(BF)  # [256, 66] permuted rows

    xT_pad = np.zeros((IN_F, PADN), dtype=BF)
    xT_pad[:, :N_NODES] = x.T.astype(BF)

    ident = np.eye(128, dtype=BF)
    b0r = np.tile(b0[il][None, :], (P, 1)).astype(BF)
    b1r = np.tile(b1[None, :], (P, 1)).astype(np.float32)
    has_b0 = bool(np.any(b0))
    has_b1 = bool(np.any(b1))

    # ---- K1: node tables
    nc1 = build_k1()
    in1 = [
        {"xT_own": np.ascontiguousarray(xT_pad[:, k * OWN : (k + 1) * OWN]), "w0te": w0te}
        for k in range(NC_CORES)
    ]
    r1 = _run(nc1, in1, "k1")
    hbtab = np.concatenate([r1[k]["hb"] for k in range(NC_CORES)], axis=0)   # [PADN,256] bf16
    elrtab = np.concatenate([r1[k]["elr"] for k in range(NC_CORES)], axis=0)  # [PADN,8] f32

    # ---- edge layout
    C, sidx, ddst, dloc, min_deg = _edge_slots(src, dst)
    use_eps = min_deg == 0

    # pre-permute table columns to (d,h)-interleaved order once
    hbtab_il = np.ascontiguousarray(hbtab[:, il])
    hbtab_x = np.concatenate([hbtab_il, np.zeros((1, IN_F), dtype=BF)], axis=0)
    elrtab_x = np.concatenate([elrtab, np.zeros((1, 8), dtype=np.float32)], axis=0)

    s_cl = np.where(sidx < 0, PADN, sidx)
    d_cl = np.where(ddst < 0, PADN, ddst)

    # one-hot tiles: S[w, p, c*128+n] = (dstloc == n)
    def s_tiles(dl):
        oh = (dl.reshape(W_PER_CORE, C, P)[:, :, :, None]
              == np.arange(128, dtype=np.float32)[None, None, None, :])
        return np.ascontiguousarray(
            oh.transpose(0, 2, 1, 3).reshape(W_PER_CORE, P, C * 128).astype(F8H)
        )

    # ---- K2 inputs
    nc2 = build_k2(C, use_eps, has_b0)
    in2 = []
    for k in range(NC_CORES):
        h_edge = _to_tiles(hbtab_x[s_cl[k], :], C, HF)          # bf16 byte gather
        meta = np.empty((W_PER_CORE, C * P, 8), dtype=np.float32)
        meta[:, :, 0:4] = elrtab_x[s_cl[k], 0:4]
        meta[:, :, 4:8] = elrtab_x[d_cl[k], 4:8]
        meta = _to_tiles(meta, C, 8).astype(BF)
        in2.append(
            {
                "h_edge": np.ascontiguousarray(h_edge),
                "meta": np.ascontiguousarray(meta),
                "S_in": s_tiles(dloc[k]),
                "b0r": b0r,
                "ident": ident,
                "w1te": w1te,
            }
        )
    r2 = _run(nc2, in2, "k2")
    gbtab = np.concatenate([r2[k]["gb"] for k in range(NC_CORES)], axis=0)   # [PADN,64] bf16
    e1tab = np.concatenate([r2[k]["e1"] for k in range(NC_CORES)], axis=0)   # [PADN,2] f32
    gbtab_x = np.concatenate([gbtab, np.zeros((1, OUT_F), dtype=BF)], axis=0)
    e1tab_x = np.concatenate([e1tab, np.zeros((1, 2), dtype=np.float32)], axis=0)

    # ---- K3 inputs
    nc3 = build_k3(C, use_eps, has_b1)
    in3 = []
    for k in range(NC_CORES):
        g_edge = _to_tiles(gbtab_x[s_cl[k], :], C, OUT_F)       # bf16 byte gather
        meta1 = np.empty((W_PER_CORE, C * P, 2), dtype=np.float32)
        meta1[:, :, 0] = e1tab_x[s_cl[k], 0]
        meta1[:, :, 1] = e1tab_x[d_cl[k], 1]
        meta1 = _to_tiles(meta1, C, 2).astype(BF)
        in3.append(
            {
                "g_edge": np.ascontiguousarray(g_edge),
                "meta1": np.ascontiguousarray(meta1),
                "S_in": in2[k]["S_in"],
                "b1r": b1r,
            }
        )
    r3 = _run(nc3, in3, "k3")
    y = np.concatenate([r3[k]["y_out"] for k in range(NC_CORES)], axis=0)
    return np.ascontiguousarray(y[:N_NODES]).astype(np.float32)
